# revision 2
# baseline (speedup 1.0000x reference)
"""Trainium2 Bass kernel for nn_MultiHeadAttention_65352222376626.

Reference computation (B=8, S=1024, D=768, H=12):
    q = einsum('bsd,hde->bhse', x, Wq) + bq      # per-head full-width projections
    k, v likewise
    scores = einsum('bhse,bhte->bhst', q, k) * sqrt(64)
    attn = softmax(scores, -1)
    o = einsum('bhst,bhte->bhse', attn, v)
    out = concat_heads(o) @ Wp + bp

Sharding: pure batch-parallel - B == n_cores == 8, one batch element per
NeuronCore, full weights replicated per core.  No collectives needed.

Algebraic restructure (v2): since softmax is row-shift invariant,
    scores = x (Wq Wk^T) x^T + ones_s (x Wk bq)^T   [+ row-const terms dropped]
so we precompute M_h = Wq_h @ Wk_h^T and r_h = Wk_h @ bq_h on the host and
replace the q-proj + k-proj + scores pipeline (3 big matmuls) with
u = x@M + r followed by scores = u @ x^T (2 big matmuls).
All device matmuls run single-pass fp16 (11-bit mantissa; PE upconverts to
FP22 and accumulates fp32).  bk shifts score rows by a constant and cancels
in softmax; bv's contribution is sum_h bv_h @ Wp_h (softmax rows sum to 1),
folded with bp into one host-side bias add.
"""

import numpy as np

B, S, D, H = 8, 1024, 768, 12
P = 128
SD = S // P   # 8 tiles along the sequence axis
ED = D // P   # 6 tiles along the feature axis
SCALE = 8.0   # sqrt(head_dim=64); reference multiplies scores by this

_CACHE = {}


def _build_nc(n_heads=H):
    import concourse.tile as tile
    from concourse import bacc, mybir
    from concourse.masks import make_identity

    f32 = mybir.dt.float32
    f16 = mybir.dt.float16
    AF = mybir.ActivationFunctionType

    nc = bacc.Bacc()

    # ---- DRAM I/O (per core) ----
    xT_d = nc.dram_tensor("xT", [D, S], f16, kind="ExternalInput")
    m_d = nc.dram_tensor("m", [H, D, D], f16, kind="ExternalInput")
    wv_d = nc.dram_tensor("wv", [H, D, D], f16, kind="ExternalInput")
    wp_d = nc.dram_tensor("wp", [H, D, D], f16, kind="ExternalInput")
    r_d = nc.dram_tensor("r", [H, D], f32, kind="ExternalInput")
    out_d = nc.dram_tensor("out", [S, D], f32, kind="ExternalOutput")

    # partition-tiled DRAM views
    xT_t = xT_d.rearrange("(o p) s -> p o s", p=P)        # [128, ED, S]
    m_t = m_d.rearrange("h (o p) e -> h p o e", p=P)      # [H, 128, ED, D]
    wv_t = wv_d.rearrange("h (o p) e -> h p o e", p=P)
    wp_t = wp_d.rearrange("h (o p) e -> h p o e", p=P)
    r_t = r_d.rearrange("h (o p) -> p h o", p=P)          # [128, H, ED]
    out_t = out_d.rearrange("(o p) d -> p o d", p=P)      # [128, SD, D]

    with tile.TileContext(nc) as tc:
        with (
            tc.tile_pool(name="persist", bufs=1) as persist,
            tc.tile_pool(name="whead", bufs=2) as whead,
            tc.tile_pool(name="work", bufs=2) as work,
            tc.tile_pool(name="small", bufs=4) as small,
            tc.tile_pool(name="mmps", bufs=2, space="PSUM") as mmps,
            tc.tile_pool(name="tpps", bufs=2, space="PSUM") as tpps,
            tc.tile_pool(name="scps", bufs=2, space="PSUM") as scps,
        ):
            # ---- persistent tiles ----
            xsb = persist.tile([P, ED, S], f16)
            nc.sync.dma_start(xsb[:], xT_t)
            rsb = persist.tile([P, H, ED], f32)
            nc.sync.dma_start(rsb[:], r_t)
            ident = persist.tile([P, P], f16)
            make_identity(nc, ident)
            acc = persist.tile([P, SD, D], f32)     # final accumulator

            mh_next = [whead.tile([P, ED, D], f16, tag="m")]
            nc.sync.dma_start(mh_next[0][:], m_t[0])

            for h in range(n_heads):
                mh = mh_next[0]
                wvh = whead.tile([P, ED, D], f16, tag="wv")
                nc.sync.dma_start(wvh[:], wv_t[h])
                wph = whead.tile([P, ED, D], f16, tag="wp")
                nc.sync.dma_start(wph[:], wp_t[h])

                # ---- u projection: uT[e,s] = sum_d M[d,e] xT[d,s] + r[e] ----
                uT = work.tile([P, ED, S], f16, tag="u", bufs=1)
                for et in range(ED):
                    e_sl = slice(et * P, (et + 1) * P)
                    for sc_ in range(2):
                        s_sl = slice(sc_ * 512, (sc_ + 1) * 512)
                        ps = mmps.tile([P, 512], f32, tag="mm512")
                        for dt_ in range(ED):
                            nc.tensor.matmul(
                                ps[:], mh[:, dt_, e_sl], xsb[:, dt_, s_sl],
                                start=(dt_ == 0), stop=(dt_ == ED - 1))
                        nc.vector.tensor_scalar_add(
                            uT[:, et, s_sl], ps[:], rsb[:, h, et:et + 1])

                # prefetch next head's M during this head's compute
                if h + 1 < n_heads:
                    mh_next[0] = whead.tile([P, ED, D], f16, tag="m")
                    nc.sync.dma_start(mh_next[0][:], m_t[h + 1])

                # ---- v projection: v[t,n] = sum_d xT[d,t] Wv[d,n] ----
                vsb = work.tile([P, SD, D], f16, tag="v", bufs=1)
                for tt in range(SD):
                    t_sl = slice(tt * P, (tt + 1) * P)
                    for (n0, n1) in ((0, 512), (512, 768)):
                        ps = mmps.tile([P, 512], f32, tag="mm512")
                        for dt_ in range(ED):
                            nc.tensor.matmul(
                                ps[:, :n1 - n0], xsb[:, dt_, t_sl],
                                wvh[:, dt_, n0:n1],
                                start=(dt_ == 0), stop=(dt_ == ED - 1))
                        nc.scalar.activation(
                            vsb[:, tt, n0:n1], ps[:, :n1 - n0], AF.Copy)

                # ---- scores + softmax; transposes pipelined one s-tile behind
                pT = work.tile([P, SD, S], f16, tag="pT", bufs=1)

                def emit_transposes(st, ptile):
                    s_sl = slice(st * P, (st + 1) * P)
                    for g in range(2):
                        tp_ps = tpps.tile([P, 4, P], f16, tag="tp")
                        for k in range(4):
                            tt = g * 4 + k
                            nc.tensor.transpose(
                                tp_ps[:, k, :], ptile[:, tt * P:(tt + 1) * P],
                                ident[:])
                        nc.vector.tensor_copy(
                            pT[:, g * 4:(g + 1) * 4, s_sl], tp_ps[:])

                pending = []
                for st in range(SD):
                    s_sl = slice(st * P, (st + 1) * P)
                    sc_ps = scps.tile([P, S], f32, tag="sc")
                    for tch in range(2):
                        t_sl = slice(tch * 512, (tch + 1) * 512)
                        for et in range(ED):
                            nc.tensor.matmul(
                                sc_ps[:, t_sl], uT[:, et, s_sl],
                                xsb[:, et, t_sl],
                                start=(et == 0), stop=(et == ED - 1))
                    negmax = small.tile([P, 1], f32, tag="negmax")
                    nc.vector.tensor_reduce(
                        negmax[:], sc_ps[:], axis=mybir.AxisListType.X,
                        op=mybir.AluOpType.max, negate=True)
                    bias8 = small.tile([P, 1], f32, tag="bias8")
                    nc.vector.tensor_scalar_mul(bias8[:], negmax[:], SCALE)
                    ptile = work.tile([P, S], f16, tag="p")
                    sumexp = small.tile([P, 1], f32, tag="sumexp")
                    nc.scalar.activation(
                        ptile[:], sc_ps[:], AF.Exp,
                        bias=bias8[:], scale=SCALE, accum_out=sumexp[:])
                    recip = small.tile([P, 1], f32, tag="recip")
                    nc.vector.reciprocal(recip[:], sumexp[:])
                    nc.vector.tensor_scalar_mul(ptile[:], ptile[:], recip[:])
                    pending.append((st, ptile))
                    if len(pending) == 2:
                        emit_transposes(*pending.pop(0))
                emit_transposes(*pending.pop(0))

                # ---- o^T[e,s] = sum_t v[t,e] P^T[t,s] ----
                oT = work.tile([P, ED, S], f16, tag="oT", bufs=1)
                for et in range(ED):
                    e_sl = slice(et * P, (et + 1) * P)
                    for sc_ in range(2):
                        s_sl = slice(sc_ * 512, (sc_ + 1) * 512)
                        ps = mmps.tile([P, 512], f32, tag="mm512")
                        for tt in range(SD):
                            nc.tensor.matmul(
                                ps[:], vsb[:, tt, e_sl], pT[:, tt, s_sl],
                                start=(tt == 0), stop=(tt == SD - 1))
                        nc.scalar.activation(
                            oT[:, et, s_sl], ps[:], AF.Copy)

                # ---- output projection, accumulate over heads ----
                for st in range(SD):
                    s_sl = slice(st * P, (st + 1) * P)
                    for (n0, n1) in ((0, 512), (512, 768)):
                        pr = mmps.tile([P, 512], f32, tag="mm512")
                        for et in range(ED):
                            nc.tensor.matmul(
                                pr[:, :n1 - n0], oT[:, et, s_sl],
                                wph[:, et, n0:n1],
                                start=(et == 0), stop=(et == ED - 1))
                        if h == 0:
                            nc.vector.tensor_copy(
                                acc[:, st, n0:n1], pr[:, :n1 - n0])
                        else:
                            nc.vector.tensor_add(
                                out=acc[:, st, n0:n1], in0=acc[:, st, n0:n1],
                                in1=pr[:, :n1 - n0])

            for st in range(SD):
                nc.sync.dma_start(out_t[:, st, :], acc[:, st, :])

    nc.compile()
    return nc


def _get_nc():
    if "nc" not in _CACHE:
        _CACHE["nc"] = _build_nc()
    return _CACHE["nc"]


def _prepare(x, Wq, bq, Wk, bk, Wv, bv, Wp, bp):
    x = np.asarray(x, dtype=np.float32)
    Wq = np.asarray(Wq, dtype=np.float32)
    Wk = np.asarray(Wk, dtype=np.float32)
    Wv = np.asarray(Wv, dtype=np.float32)
    Wp = np.asarray(Wp, dtype=np.float32)
    bq = np.asarray(bq, dtype=np.float32)
    bv = np.asarray(bv, dtype=np.float32)
    bp = np.asarray(bp, dtype=np.float32)

    # scores = x M x^T + ones (x r)^T with M = Wq Wk^T, r = Wk bq.
    # (x Wq bk^T and bq.bk shift rows uniformly and cancel in softmax.)
    M = np.matmul(Wq, Wk.transpose(0, 2, 1))          # [H, D, D]
    r = np.matmul(Wk, bq[:, :, None])[:, :, 0]        # [H, D]
    wp3 = Wp.reshape(H, D, D)

    # bv contributes sum_h bv_h @ Wp_h to every output row (softmax rows sum
    # to 1); fold it and bp into one host-side bias.  bk dropped entirely.
    bp_eff = (bp.astype(np.float64)
              + np.einsum('hd,hde->e', bv.astype(np.float64),
                          wp3.astype(np.float64))).astype(np.float32)

    shared = {
        "m": M.astype(np.float16),
        "wv": Wv.astype(np.float16),
        "wp": wp3.astype(np.float16),
        "r": r.astype(np.float32),
    }
    in_maps = []
    for b in range(B):
        xT = np.ascontiguousarray(x[b].T).astype(np.float16)
        in_maps.append({"xT": xT, **shared})
    return in_maps, bp_eff


def kernel(x, Wq, bq, Wk, bk, Wv, bv, Wp, bp):
    from concourse.bass_utils import run_bass_kernel_spmd

    in_maps, bp_eff = _prepare(x, Wq, bq, Wk, bk, Wv, bv, Wp, bp)
    nc = _get_nc()
    res = run_bass_kernel_spmd(nc, in_maps, list(range(B)))
    out = np.stack([res.results[b]["out"] for b in range(B)], axis=0)
    out = out + bp_eff[None, None, :]
    return out.astype(np.float32)


# revision 9
# speedup vs baseline: 1.8191x; 1.8191x over previous
"""Trainium2 Bass kernel for nn_MultiHeadAttention_65352222376626.

Reference computation (B=8, S=1024, D=768, H=12):
    q = einsum('bsd,hde->bhse', x, Wq) + bq      # per-head full-width projections
    k, v likewise
    scores = einsum('bhse,bhte->bhst', q, k) * sqrt(64)
    attn = softmax(scores, -1)
    o = einsum('bhst,bhte->bhse', attn, v)
    out = concat_heads(o) @ Wp + bp

Sharding: pure batch-parallel - B == n_cores == 8, one batch element per
NeuronCore, full weights replicated per core.  No collectives needed.

Algebraic restructure: since softmax is row-shift invariant,
    scores = x (Wq Wk^T) x^T + ones_s (x Wk bq)^T   [+ row-const terms dropped]
so we precompute M_h = Wq_h @ Wk_h^T and r_h = Wk_h @ bq_h on the host and
replace the q-proj + k-proj + scores pipeline (3 big matmuls) with
u = x@M + r followed by scores = u @ x^T (2 big matmuls).  Likewise by
associativity (P@v)@Wp = P@(x@(Wv@Wp)), so W2_h = Wv_h @ Wp_h is
precomputed and the v-proj + attn@v + out-proj trio becomes
w = x@W2 then out += P@w (2 big matmuls).

Numerics: the softmax is near-argmax (score std ~222, top-2 gaps ~60), so
logit errors flip argmaxes and blow up the absmax metric.  The u-proj and
score matmuls therefore run 3-pass fp16 hi/lo (hi*hi + lo*hi + hi*lo,
~2^-22 operand precision); the w/out path is tolerance-insensitive and
runs single-pass fp16.  bk shifts score rows by a constant and cancels in
softmax; bv's contribution is sum_h bv_h @ Wp_h (softmax rows sum to 1),
folded with bp into one host-side bias add.
"""

import numpy as np
import ml_dtypes

B, S, D, H = 8, 1024, 768, 12
P = 128
SD = S // P   # 8 tiles along the sequence axis
ED = D // P   # 6 tiles along the feature axis
SCALE = 8.0   # sqrt(head_dim=64); reference multiplies scores by this

_CACHE = {}


def _build_nc(n_heads=H):
    import concourse.tile as tile
    from concourse import bacc, mybir
    from concourse.masks import make_identity

    f32 = mybir.dt.float32
    f16 = mybir.dt.float16
    AF = mybir.ActivationFunctionType

    nc = bacc.Bacc()

    # ---- DRAM I/O (per core) ----
    xh_d = nc.dram_tensor("xT_hi", [D, S], f16, kind="ExternalInput")
    xl_d = nc.dram_tensor("xT_lo", [D, S], f16, kind="ExternalInput")
    mh_d = nc.dram_tensor("m_hi", [H, D, D], f16, kind="ExternalInput")
    ml_d = nc.dram_tensor("m_lo", [H, D, D], f16, kind="ExternalInput")
    w2_d = nc.dram_tensor("w2", [H, D, D], f16, kind="ExternalInput")
    r_d = nc.dram_tensor("r", [H, D], f32, kind="ExternalInput")
    out_d = nc.dram_tensor("out", [S, D], f32, kind="ExternalOutput")

    # partition-tiled DRAM views
    xh_t = xh_d.rearrange("(o p) s -> p o s", p=P)        # [128, ED, S]
    xl_t = xl_d.rearrange("(o p) s -> p o s", p=P)
    mh_t = mh_d.rearrange("h (o p) e -> h p o e", p=P)    # [H, 128, ED, D]
    ml_t = ml_d.rearrange("h (o p) e -> h p o e", p=P)
    w2_t = w2_d.rearrange("h (o p) e -> h p o e", p=P)
    r_t = r_d.rearrange("h (o p) -> p h o", p=P)          # [128, H, ED]
    out_t = out_d.rearrange("(o p) d -> p o d", p=P)      # [128, SD, D]

    with tile.TileContext(nc) as tc:
        with (
            tc.tile_pool(name="persist", bufs=1) as persist,
            tc.tile_pool(name="whead", bufs=2) as whead,
            tc.tile_pool(name="work", bufs=2) as work,
            tc.tile_pool(name="small", bufs=4) as small,
            tc.tile_pool(name="mmps", bufs=2, space="PSUM") as mmps,
            tc.tile_pool(name="tpps", bufs=2, space="PSUM") as tpps,
            tc.tile_pool(name="scps", bufs=2, space="PSUM") as scps,
        ):
            # ---- persistent tiles ----
            xh = persist.tile([P, ED, S], f16)
            nc.sync.dma_start(xh[:], xh_t)
            xl = persist.tile([P, ED, S], f16)
            nc.sync.dma_start(xl[:], xl_t)
            rsb = persist.tile([P, H, ED], f32)
            nc.sync.dma_start(rsb[:], r_t)
            ident = persist.tile([P, P], f16)
            make_identity(nc, ident)
            acc = persist.tile([P, SD, D], f32)     # final accumulator

            mnext = [whead.tile([P, ED, D], f16, tag="mh", name="mhi0"),
                     whead.tile([P, ED, D], f16, tag="ml", name="mlo0")]
            nc.sync.dma_start(mnext[0][:], mh_t[0])
            nc.sync.dma_start(mnext[1][:], ml_t[0])

            for h in range(n_heads):
                mhi, mlo = mnext
                w2h = whead.tile([P, ED, D], f16, tag="w2")
                nc.sync.dma_start(w2h[:], w2_t[h])

                # ---- u proj (3-pass): uT[e,s] = sum_d M[d,e] xT[d,s] + r[e]
                uh = work.tile([P, ED, S], f16, tag="uh", bufs=1)
                ul = work.tile([P, ED, S], f16, tag="ul", bufs=1)
                for et in range(ED):
                    e_sl = slice(et * P, (et + 1) * P)
                    for sc_ in range(2):
                        s_sl = slice(sc_ * 512, (sc_ + 1) * 512)
                        ps = mmps.tile([P, 512], f32, tag="mm512")
                        for dt_ in range(ED):
                            nc.tensor.matmul(
                                ps[:], mhi[:, dt_, e_sl], xh[:, dt_, s_sl],
                                start=(dt_ == 0), stop=False)
                            nc.tensor.matmul(
                                ps[:], mlo[:, dt_, e_sl], xh[:, dt_, s_sl],
                                start=False, stop=False)
                            nc.tensor.matmul(
                                ps[:], mhi[:, dt_, e_sl], xl[:, dt_, s_sl],
                                start=False, stop=(dt_ == ED - 1))
                        # u += r (per-partition), then split to fp16 hi/lo
                        nc.vector.tensor_scalar_add(
                            ps[:], ps[:], rsb[:, h, et:et + 1])
                        nc.scalar.activation(uh[:, et, s_sl], ps[:], AF.Copy)
                        nc.vector.tensor_sub(
                            ul[:, et, s_sl], ps[:], uh[:, et, s_sl])

                # prefetch next head's M during this head's compute
                if h + 1 < n_heads:
                    mnext = [
                        whead.tile([P, ED, D], f16, tag="mh",
                                   name=f"mhi{h + 1}"),
                        whead.tile([P, ED, D], f16, tag="ml",
                                   name=f"mlo{h + 1}")]
                    nc.sync.dma_start(mnext[0][:], mh_t[h + 1])
                    nc.sync.dma_start(mnext[1][:], ml_t[h + 1])

                # ---- w projection (1-pass): w[t,n] = sum_d xT[d,t] W2[d,n]
                wsb = work.tile([P, SD, D], f16, tag="w", bufs=1)
                for tt in range(SD):
                    t_sl = slice(tt * P, (tt + 1) * P)
                    for (n0, n1) in ((0, 512), (512, 768)):
                        ps = mmps.tile([P, 512], f32, tag="mm512")
                        for dt_ in range(ED):
                            nc.tensor.matmul(
                                ps[:, :n1 - n0], xh[:, dt_, t_sl],
                                w2h[:, dt_, n0:n1],
                                start=(dt_ == 0), stop=(dt_ == ED - 1))
                        nc.scalar.activation(
                            wsb[:, tt, n0:n1], ps[:, :n1 - n0], AF.Copy)

                # ---- scores (3-pass) + softmax; transposes one s-tile behind
                pT = work.tile([P, SD, S], f16, tag="pT", bufs=1)

                def emit_transposes(st, ptile):
                    s_sl = slice(st * P, (st + 1) * P)
                    for g in range(2):
                        tp_ps = tpps.tile([P, 4, P], f16, tag="tp")
                        for k in range(4):
                            tt = g * 4 + k
                            nc.tensor.transpose(
                                tp_ps[:, k, :], ptile[:, tt * P:(tt + 1) * P],
                                ident[:])
                        nc.vector.tensor_copy(
                            pT[:, g * 4:(g + 1) * 4, s_sl], tp_ps[:])

                pending = []
                for st in range(SD):
                    s_sl = slice(st * P, (st + 1) * P)
                    sc_ps = scps.tile([P, S], f32, tag="sc")
                    for tch in range(2):
                        t_sl = slice(tch * 512, (tch + 1) * 512)
                        for et in range(ED):
                            nc.tensor.matmul(
                                sc_ps[:, t_sl], uh[:, et, s_sl],
                                xh[:, et, t_sl],
                                start=(et == 0), stop=False)
                            nc.tensor.matmul(
                                sc_ps[:, t_sl], ul[:, et, s_sl],
                                xh[:, et, t_sl],
                                start=False, stop=False)
                            nc.tensor.matmul(
                                sc_ps[:, t_sl], uh[:, et, s_sl],
                                xl[:, et, t_sl],
                                start=False, stop=(et == ED - 1))
                    negmax = small.tile([P, 1], f32, tag="negmax")
                    nc.vector.tensor_reduce(
                        negmax[:], sc_ps[:], axis=mybir.AxisListType.X,
                        op=mybir.AluOpType.max, negate=True)
                    bias8 = small.tile([P, 1], f32, tag="bias8")
                    nc.vector.tensor_scalar_mul(bias8[:], negmax[:], SCALE)
                    ptile = work.tile([P, S], f16, tag="p")
                    sumexp = small.tile([P, 1], f32, tag="sumexp")
                    nc.scalar.activation(
                        ptile[:], sc_ps[:], AF.Exp,
                        bias=bias8[:], scale=SCALE, accum_out=sumexp[:])
                    recip = small.tile([P, 1], f32, tag="recip")
                    nc.vector.reciprocal(recip[:], sumexp[:])
                    nc.vector.tensor_scalar_mul(ptile[:], ptile[:], recip[:])
                    pending.append((st, ptile))
                    if len(pending) == 2:
                        emit_transposes(*pending.pop(0))
                emit_transposes(*pending.pop(0))

                # ---- out[s,n] += sum_t P[s,t] w[t,n], accumulated over heads
                for st in range(SD):
                    s_sl = slice(st * P, (st + 1) * P)
                    for (n0, n1) in ((0, 512), (512, 768)):
                        pr = mmps.tile([P, 512], f32, tag="mm512")
                        for tt in range(SD):
                            nc.tensor.matmul(
                                pr[:, :n1 - n0], pT[:, tt, s_sl],
                                wsb[:, tt, n0:n1],
                                start=(tt == 0), stop=(tt == SD - 1))
                        if h == 0:
                            nc.vector.tensor_copy(
                                acc[:, st, n0:n1], pr[:, :n1 - n0])
                        else:
                            nc.vector.tensor_add(
                                out=acc[:, st, n0:n1], in0=acc[:, st, n0:n1],
                                in1=pr[:, :n1 - n0])

            for st in range(SD):
                nc.sync.dma_start(out_t[:, st, :], acc[:, st, :])

    nc.compile()
    return nc


def _get_nc():
    if "nc" not in _CACHE:
        _CACHE["nc"] = _build_nc()
    return _CACHE["nc"]


def _split_f16(a32):
    hi = a32.astype(np.float16)
    lo = (a32 - hi.astype(np.float32)).astype(np.float16)
    return hi, lo


def _prepare(x, Wq, bq, Wk, bk, Wv, bv, Wp, bp):
    x = np.asarray(x, dtype=np.float32)
    Wq = np.asarray(Wq, dtype=np.float32)
    Wk = np.asarray(Wk, dtype=np.float32)
    Wv = np.asarray(Wv, dtype=np.float32)
    Wp = np.asarray(Wp, dtype=np.float32)
    bq = np.asarray(bq, dtype=np.float32)
    bv = np.asarray(bv, dtype=np.float32)
    bp = np.asarray(bp, dtype=np.float32)

    # scores = x M x^T + ones (x r)^T with M = Wq Wk^T, r = Wk bq.
    # (x Wq bk^T and bq.bk shift rows uniformly and cancel in softmax.)
    M = np.matmul(Wq, Wk.transpose(0, 2, 1))          # [H, D, D]
    r = np.matmul(Wk, bq[:, :, None])[:, :, 0]        # [H, D]
    wp3 = Wp.reshape(H, D, D)
    W2 = np.matmul(Wv, wp3)                           # [H, D, D]

    # bv contributes sum_h bv_h @ Wp_h to every output row (softmax rows sum
    # to 1); fold it and bp into one host-side bias.  bk dropped entirely.
    bp_eff = (bp.astype(np.float64)
              + np.einsum('hd,hde->e', bv.astype(np.float64),
                          wp3.astype(np.float64))).astype(np.float32)

    m_hi, m_lo = _split_f16(M)
    shared = {
        "m_hi": m_hi, "m_lo": m_lo,
        "w2": W2.astype(np.float16),
        "r": r.astype(np.float32),
    }
    in_maps = []
    for b in range(B):
        xT = np.ascontiguousarray(x[b].T)
        xt_hi, xt_lo = _split_f16(xT)
        in_maps.append({"xT_hi": xt_hi, "xT_lo": xt_lo, **shared})
    return in_maps, bp_eff


def kernel(x, Wq, bq, Wk, bk, Wv, bv, Wp, bp):
    from concourse.bass_utils import run_bass_kernel_spmd

    in_maps, bp_eff = _prepare(x, Wq, bq, Wk, bk, Wv, bv, Wp, bp)
    nc = _get_nc()
    res = run_bass_kernel_spmd(nc, in_maps, list(range(B)))
    out = np.stack([res.results[b]["out"] for b in range(B)], axis=0)
    out = out + bp_eff[None, None, :]
    return out.astype(np.float32)


# revision 10
# speedup vs baseline: 2.2467x; 1.2350x over previous
"""Trainium2 Bass kernel for nn_MultiHeadAttention_65352222376626.

Reference computation (B=8, S=1024, D=768, H=12):
    q = einsum('bsd,hde->bhse', x, Wq) + bq      # per-head full-width projections
    k, v likewise
    scores = einsum('bhse,bhte->bhst', q, k) * sqrt(64)
    attn = softmax(scores, -1)
    o = einsum('bhst,bhte->bhse', attn, v)
    out = concat_heads(o) @ Wp + bp

Sharding: pure batch-parallel - B == n_cores == 8, one batch element per
NeuronCore, full weights replicated per core.  No collectives needed.

Algebraic restructure: since softmax is row-shift invariant,
    scores = x (Wq Wk^T) x^T + ones_s (x Wk bq)^T   [+ row-const terms dropped]
so we precompute M_h = Wq_h @ Wk_h^T and r_h = Wk_h @ bq_h on the host and
replace the q-proj + k-proj + scores pipeline (3 big matmuls) with
u = x@M + r followed by scores = u @ x^T (2 big matmuls).  Likewise by
associativity (P@v)@Wp = P@(x@(Wv@Wp)), so W2_h = Wv_h @ Wp_h is
precomputed and the v-proj + attn@v + out-proj trio becomes
w = x@W2 then out += P@w (2 big matmuls).

Numerics: the softmax is near-argmax (score std ~222, top-2 gaps ~60), so
logit errors flip argmaxes and blow up the absmax metric; the u-proj and
score matmul operands need ~16+ mantissa bits.  Each runs as one fp16
hi*hi pass plus ONE fp8-e5m2 DoubleRow matmul that computes both
correction terms (lo*full + hi*lo) as a K-interleaved pair at 2x rate -
1.5 effective passes instead of 3.  The correction terms are ~2^-11 of
the main term, so 3-bit e5m2 mantissas suffice (validated: absmax rel
err 2.3e-3 vs fp32 reference).  M is pre-scaled by 32 on the host so its
fp8 splits stay in e5m2's normal range; the 1/32 is folded into the
PSUM->SBUF epilogue.  The w/out path is tolerance-insensitive and runs
single-pass fp16.  bk shifts score rows by a constant and cancels in
softmax; bv's contribution is sum_h bv_h @ Wp_h (softmax rows sum to 1),
folded with bp into one host-side bias add.
"""

import numpy as np
import ml_dtypes

B, S, D, H = 8, 1024, 768, 12
P = 128
SD = S // P   # 8 tiles along the sequence axis
ED = D // P   # 6 tiles along the feature axis
SCALE = 8.0   # sqrt(head_dim=64); reference multiplies scores by this
MSCALE = 32.0  # host pre-scale of M so fp8 splits stay in e5m2 range

_CACHE = {}


def _build_nc(n_heads=H):
    import concourse.tile as tile
    from concourse import bacc, mybir
    from concourse.masks import make_identity

    f32 = mybir.dt.float32
    f16 = mybir.dt.float16
    f8 = mybir.dt.float8e5
    AF = mybir.ActivationFunctionType
    ALU = mybir.AluOpType
    DR = mybir.MatmulPerfMode.DoubleRow

    nc = bacc.Bacc()

    # ---- DRAM I/O (per core) ----
    xh_d = nc.dram_tensor("xT_hi", [D, S], f16, kind="ExternalInput")
    x8_d = nc.dram_tensor("x8", [D, 2, S], f8, kind="ExternalInput")
    mh_d = nc.dram_tensor("m_hi", [H, D, D], f16, kind="ExternalInput")
    m8_d = nc.dram_tensor("m8", [H, D, 2, D], f8, kind="ExternalInput")
    w2_d = nc.dram_tensor("w2", [H, D, D], f16, kind="ExternalInput")
    r_d = nc.dram_tensor("r", [H, D], f32, kind="ExternalInput")
    out_d = nc.dram_tensor("out", [S, D], f32, kind="ExternalOutput")

    # partition-tiled DRAM views
    xh_t = xh_d.rearrange("(o p) s -> p o s", p=P)          # [128, ED, S]
    x8_t = x8_d.rearrange("(o p) t s -> p o t s", p=P)      # [128, ED, 2, S]
    mh_t = mh_d.rearrange("h (o p) e -> h p o e", p=P)      # [H, 128, ED, D]
    m8_t = m8_d.rearrange("h (o p) t e -> h p o t e", p=P)  # [H, 128, ED, 2, D]
    w2_t = w2_d.rearrange("h (o p) e -> h p o e", p=P)
    r_t = r_d.rearrange("h (o p) -> p h o", p=P)            # [128, H, ED]
    out_t = out_d.rearrange("(o p) d -> p o d", p=P)        # [128, SD, D]

    with tile.TileContext(nc) as tc:
        with (
            tc.tile_pool(name="persist", bufs=1) as persist,
            tc.tile_pool(name="whead", bufs=2) as whead,
            tc.tile_pool(name="work", bufs=2) as work,
            tc.tile_pool(name="small", bufs=4) as small,
            tc.tile_pool(name="mmps", bufs=2, space="PSUM") as mmps,
            tc.tile_pool(name="tpps", bufs=2, space="PSUM") as tpps,
            tc.tile_pool(name="scps", bufs=2, space="PSUM") as scps,
        ):
            # ---- persistent tiles ----
            xh = persist.tile([P, ED, S], f16)
            nc.sync.dma_start(xh[:], xh_t)
            x8 = persist.tile([P, ED, 2, S], f8)
            nc.sync.dma_start(x8[:], x8_t)
            rsb = persist.tile([P, H, ED], f32)
            nc.sync.dma_start(rsb[:], r_t)
            ident = persist.tile([P, P], f16)
            make_identity(nc, ident)
            acc = persist.tile([P, SD, D], f32)     # final accumulator

            mnext = [whead.tile([P, ED, D], f16, tag="mh", name="mhi0"),
                     whead.tile([P, ED, 2, D], f8, tag="m8", name="m8_0")]
            nc.sync.dma_start(mnext[0][:], mh_t[0])
            nc.sync.dma_start(mnext[1][:], m8_t[0])

            for h in range(n_heads):
                mhi, m8h = mnext
                w2h = whead.tile([P, ED, D], f16, tag="w2")
                nc.sync.dma_start(w2h[:], w2_t[h])

                # ---- u proj: uT[e,s] = (sum_d 32M[d,e] xT[d,s] + 32r[e])/32
                # fp16 hi*hi pass + one DoubleRow fp8 pass for (lo*full +
                # hi*lo); epilogue writes fp16 hi + fp8 (lo, hi) operand set
                # for the score matmul.
                uh = work.tile([P, ED, S], f16, tag="uh", bufs=1)
                u8 = work.tile([P, ED, 2, S], f8, tag="u8", bufs=1)
                for et in range(ED):
                    e_sl = slice(et * P, (et + 1) * P)
                    for sc_ in range(2):
                        s_sl = slice(sc_ * 512, (sc_ + 1) * 512)
                        ps = mmps.tile([P, 512], f32, tag="mm512")
                        for dt_ in range(ED):
                            nc.tensor.matmul(
                                ps[:], mhi[:, dt_, e_sl], xh[:, dt_, s_sl],
                                start=(dt_ == 0), stop=False)
                        for dt_ in range(ED):
                            nc.tensor.matmul(
                                ps[:], m8h[:, dt_, :, e_sl],
                                x8[:, dt_, :, s_sl],
                                start=False, stop=(dt_ == ED - 1),
                                perf_mode=DR)
                        # ps = (ps + 32r)/32, then split: fp16 hi + fp8 pair
                        nc.vector.tensor_scalar(
                            ps[:], ps[:], rsb[:, h, et:et + 1], 1.0 / MSCALE,
                            op0=ALU.add, op1=ALU.mult)
                        nc.scalar.activation(uh[:, et, s_sl], ps[:], AF.Copy)
                        nc.scalar.activation(
                            u8[:, et, 1, s_sl], ps[:], AF.Copy)
                        nc.vector.tensor_sub(
                            u8[:, et, 0, s_sl], ps[:], uh[:, et, s_sl])

                # prefetch next head's M during this head's compute
                if h + 1 < n_heads:
                    mnext = [
                        whead.tile([P, ED, D], f16, tag="mh",
                                   name=f"mhi{h + 1}"),
                        whead.tile([P, ED, 2, D], f8, tag="m8",
                                   name=f"m8_{h + 1}")]
                    nc.sync.dma_start(mnext[0][:], mh_t[h + 1])
                    nc.sync.dma_start(mnext[1][:], m8_t[h + 1])

                # ---- w projection (1-pass): w[t,n] = sum_d xT[d,t] W2[d,n]
                wsb = work.tile([P, SD, D], f16, tag="w", bufs=1)
                for tt in range(SD):
                    t_sl = slice(tt * P, (tt + 1) * P)
                    for (n0, n1) in ((0, 512), (512, 768)):
                        ps = mmps.tile([P, 512], f32, tag="mm512")
                        for dt_ in range(ED):
                            nc.tensor.matmul(
                                ps[:, :n1 - n0], xh[:, dt_, t_sl],
                                w2h[:, dt_, n0:n1],
                                start=(dt_ == 0), stop=(dt_ == ED - 1))
                        nc.scalar.activation(
                            wsb[:, tt, n0:n1], ps[:, :n1 - n0], AF.Copy)

                # ---- scores + softmax; transposes one s-tile behind ----
                pT = work.tile([P, SD, S], f16, tag="pT", bufs=1)

                def emit_transposes(st, ptile):
                    s_sl = slice(st * P, (st + 1) * P)
                    for g in range(2):
                        tp_ps = tpps.tile([P, 4, P], f16, tag="tp")
                        for k in range(4):
                            tt = g * 4 + k
                            nc.tensor.transpose(
                                tp_ps[:, k, :], ptile[:, tt * P:(tt + 1) * P],
                                ident[:])
                        nc.vector.tensor_copy(
                            pT[:, g * 4:(g + 1) * 4, s_sl], tp_ps[:])

                pending = []
                for st in range(SD):
                    s_sl = slice(st * P, (st + 1) * P)
                    sc_ps = scps.tile([P, S], f32, tag="sc")
                    for tch in range(2):
                        t_sl = slice(tch * 512, (tch + 1) * 512)
                        for et in range(ED):
                            nc.tensor.matmul(
                                sc_ps[:, t_sl], uh[:, et, s_sl],
                                xh[:, et, t_sl],
                                start=(et == 0), stop=False)
                        for et in range(ED):
                            nc.tensor.matmul(
                                sc_ps[:, t_sl], u8[:, et, :, s_sl],
                                x8[:, et, :, t_sl],
                                start=False, stop=(et == ED - 1),
                                perf_mode=DR)
                    negmax = small.tile([P, 1], f32, tag="negmax")
                    nc.vector.tensor_reduce(
                        negmax[:], sc_ps[:], axis=mybir.AxisListType.X,
                        op=mybir.AluOpType.max, negate=True)
                    bias8 = small.tile([P, 1], f32, tag="bias8")
                    nc.vector.tensor_scalar_mul(bias8[:], negmax[:], SCALE)
                    ptile = work.tile([P, S], f16, tag="p")
                    sumexp = small.tile([P, 1], f32, tag="sumexp")
                    nc.scalar.activation(
                        ptile[:], sc_ps[:], AF.Exp,
                        bias=bias8[:], scale=SCALE, accum_out=sumexp[:])
                    recip = small.tile([P, 1], f32, tag="recip")
                    nc.vector.reciprocal(recip[:], sumexp[:])
                    nc.vector.tensor_scalar_mul(ptile[:], ptile[:], recip[:])
                    pending.append((st, ptile))
                    if len(pending) == 2:
                        emit_transposes(*pending.pop(0))
                emit_transposes(*pending.pop(0))

                # ---- out[s,n] += sum_t P[s,t] w[t,n], accumulated over heads
                for st in range(SD):
                    s_sl = slice(st * P, (st + 1) * P)
                    for (n0, n1) in ((0, 512), (512, 768)):
                        pr = mmps.tile([P, 512], f32, tag="mm512")
                        for tt in range(SD):
                            nc.tensor.matmul(
                                pr[:, :n1 - n0], pT[:, tt, s_sl],
                                wsb[:, tt, n0:n1],
                                start=(tt == 0), stop=(tt == SD - 1))
                        if h == 0:
                            nc.vector.tensor_copy(
                                acc[:, st, n0:n1], pr[:, :n1 - n0])
                        else:
                            nc.vector.tensor_add(
                                out=acc[:, st, n0:n1], in0=acc[:, st, n0:n1],
                                in1=pr[:, :n1 - n0])

            for st in range(SD):
                nc.sync.dma_start(out_t[:, st, :], acc[:, st, :])

    nc.compile()
    return nc


def _get_nc():
    if "nc" not in _CACHE:
        _CACHE["nc"] = _build_nc()
    return _CACHE["nc"]


def _prepare(x, Wq, bq, Wk, bk, Wv, bv, Wp, bp):
    f16 = np.float16
    e5 = ml_dtypes.float8_e5m2
    x = np.asarray(x, dtype=np.float32)
    Wq = np.asarray(Wq, dtype=np.float32)
    Wk = np.asarray(Wk, dtype=np.float32)
    Wv = np.asarray(Wv, dtype=np.float32)
    Wp = np.asarray(Wp, dtype=np.float32)
    bq = np.asarray(bq, dtype=np.float32)
    bv = np.asarray(bv, dtype=np.float32)
    bp = np.asarray(bp, dtype=np.float32)

    # scores = x M x^T + ones (x r)^T with M = Wq Wk^T, r = Wk bq.
    # (x Wq bk^T and bq.bk shift rows uniformly and cancel in softmax.)
    M = np.matmul(Wq, Wk.transpose(0, 2, 1))          # [H, D, D]
    r = np.matmul(Wk, bq[:, :, None])[:, :, 0]        # [H, D]
    wp3 = Wp.reshape(H, D, D)
    W2 = np.matmul(Wv, wp3)                           # [H, D, D]

    # bv contributes sum_h bv_h @ Wp_h to every output row (softmax rows sum
    # to 1); fold it and bp into one host-side bias.  bk dropped entirely.
    bp_eff = (bp.astype(np.float64)
              + np.einsum('hd,hde->e', bv.astype(np.float64),
                          wp3.astype(np.float64))).astype(np.float32)

    M32 = MSCALE * M
    m_hi = M32.astype(f16)
    m8 = np.empty((H, D, 2, D), dtype=e5)
    m8[:, :, 0, :] = (M32 - m_hi.astype(np.float32)).astype(e5)  # lo (x full)
    m8[:, :, 1, :] = M32.astype(e5)                              # hi (x lo)

    shared = {
        "m_hi": m_hi, "m8": m8,
        "w2": W2.astype(f16),
        "r": (MSCALE * r).astype(np.float32),
    }
    in_maps = []
    for b in range(B):
        xT = np.ascontiguousarray(x[b].T)
        xt_hi = xT.astype(f16)
        x8 = np.empty((D, 2, S), dtype=e5)
        x8[:, 0, :] = xT.astype(e5)                              # full
        x8[:, 1, :] = (xT - xt_hi.astype(np.float32)).astype(e5)  # lo
        in_maps.append({"xT_hi": xt_hi, "x8": x8, **shared})
    return in_maps, bp_eff


def kernel(x, Wq, bq, Wk, bk, Wv, bv, Wp, bp):
    from concourse.bass_utils import run_bass_kernel_spmd

    in_maps, bp_eff = _prepare(x, Wq, bq, Wk, bk, Wv, bv, Wp, bp)
    nc = _get_nc()
    res = run_bass_kernel_spmd(nc, in_maps, list(range(B)))
    out = np.stack([res.results[b]["out"] for b in range(B)], axis=0)
    out = out + bp_eff[None, None, :]
    return out.astype(np.float32)


# revision 12
# speedup vs baseline: 2.2499x; 1.0014x over previous
"""Trainium2 Bass kernel for nn_MultiHeadAttention_65352222376626.

Reference computation (B=8, S=1024, D=768, H=12):
    q = einsum('bsd,hde->bhse', x, Wq) + bq      # per-head full-width projections
    k, v likewise
    scores = einsum('bhse,bhte->bhst', q, k) * sqrt(64)
    attn = softmax(scores, -1)
    o = einsum('bhst,bhte->bhse', attn, v)
    out = concat_heads(o) @ Wp + bp

Sharding: pure batch-parallel - B == n_cores == 8, one batch element per
NeuronCore, full weights replicated per core.  No collectives needed.

Algebraic restructure: since softmax is row-shift invariant,
    scores = x (Wq Wk^T) x^T + ones_s (x Wk bq)^T   [+ row-const terms dropped]
so we precompute M_h = Wq_h @ Wk_h^T and r_h = Wk_h @ bq_h on the host and
replace the q-proj + k-proj + scores pipeline (3 big matmuls) with
u = x@M + r followed by scores = u @ x^T (2 big matmuls).  Likewise by
associativity (P@v)@Wp = P@(x@(Wv@Wp)), so W2_h = Wv_h @ Wp_h is
precomputed and the v-proj + attn@v + out-proj trio becomes
w = x@W2 then out += P@w (2 big matmuls).

Numerics: the softmax is near-argmax (score std ~222, top-2 gaps ~60), so
logit errors flip argmaxes and blow up the absmax metric; the u-proj and
score matmul operands need ~16+ mantissa bits.  Each runs as one fp16
hi*hi pass plus ONE fp8-e5m2 DoubleRow matmul that computes both
correction terms (lo*full + hi*lo) as a K-interleaved pair at 2x rate -
1.5 effective passes instead of 3.  The correction terms are ~2^-11 of
the main term, so 3-bit e5m2 mantissas suffice (validated: absmax rel
err 2.3e-3 vs fp32 reference).  M is pre-scaled by 32 on the host so its
fp8 splits stay in e5m2's normal range; the 1/32 is folded into the
PSUM->SBUF epilogue.  The w/out path is tolerance-insensitive and runs
single-pass fp16.  bk shifts score rows by a constant and cancels in
softmax; bv's contribution is sum_h bv_h @ Wp_h (softmax rows sum to 1),
folded with bp into one host-side bias add.
"""

import numpy as np
import ml_dtypes

B, S, D, H = 8, 1024, 768, 12
P = 128
SD = S // P   # 8 tiles along the sequence axis
ED = D // P   # 6 tiles along the feature axis
SCALE = 8.0   # sqrt(head_dim=64); reference multiplies scores by this
MSCALE = 32.0  # host pre-scale of M so fp8 splits stay in e5m2 range

_CACHE = {}


def _build_nc(n_heads=H):
    import concourse.tile as tile
    from concourse import bacc, mybir
    from concourse.masks import make_identity

    f32 = mybir.dt.float32
    f16 = mybir.dt.float16
    f8 = mybir.dt.float8e5
    AF = mybir.ActivationFunctionType
    ALU = mybir.AluOpType
    DR = mybir.MatmulPerfMode.DoubleRow

    nc = bacc.Bacc()

    # ---- DRAM I/O (per core) ----
    xh_d = nc.dram_tensor("xT_hi", [D, S], f16, kind="ExternalInput")
    x8_d = nc.dram_tensor("x8", [D, 2, S], f8, kind="ExternalInput")
    mh_d = nc.dram_tensor("m_hi", [H, D, D], f16, kind="ExternalInput")
    m8_d = nc.dram_tensor("m8", [H, D, 2, D], f8, kind="ExternalInput")
    w2_d = nc.dram_tensor("w2", [H, D, D], f16, kind="ExternalInput")
    r_d = nc.dram_tensor("r", [H, D], f32, kind="ExternalInput")
    out_d = nc.dram_tensor("out", [S, D], f32, kind="ExternalOutput")

    # partition-tiled DRAM views
    xh_t = xh_d.rearrange("(o p) s -> p o s", p=P)          # [128, ED, S]
    x8_t = x8_d.rearrange("(o p) t s -> p o t s", p=P)      # [128, ED, 2, S]
    mh_t = mh_d.rearrange("h (o p) e -> h p o e", p=P)      # [H, 128, ED, D]
    m8_t = m8_d.rearrange("h (o p) t e -> h p o t e", p=P)  # [H, 128, ED, 2, D]
    w2_t = w2_d.rearrange("h (o p) e -> h p o e", p=P)
    r_t = r_d.rearrange("h (o p) -> p h o", p=P)            # [128, H, ED]
    out_t = out_d.rearrange("(o p) d -> p o d", p=P)        # [128, SD, D]

    with tile.TileContext(nc) as tc:
        with (
            tc.tile_pool(name="persist", bufs=1) as persist,
            tc.tile_pool(name="whead", bufs=2) as whead,
            tc.tile_pool(name="work", bufs=2) as work,
            tc.tile_pool(name="small", bufs=4) as small,
            tc.tile_pool(name="mmps", bufs=2, space="PSUM") as mmps,
            tc.tile_pool(name="tpps", bufs=2, space="PSUM") as tpps,
            tc.tile_pool(name="scps", bufs=2, space="PSUM") as scps,
        ):
            # ---- persistent tiles ----
            xh = persist.tile([P, ED, S], f16)
            nc.sync.dma_start(xh[:], xh_t)
            x8 = persist.tile([P, ED, 2, S], f8)
            nc.sync.dma_start(x8[:], x8_t)
            rsb = persist.tile([P, H, ED], f32)
            nc.sync.dma_start(rsb[:], r_t)
            ident = persist.tile([P, P], f16)
            make_identity(nc, ident)
            acc = persist.tile([P, SD, D], f32)     # final accumulator

            mnext = [whead.tile([P, ED, D], f16, tag="mh", name="mhi0"),
                     whead.tile([P, ED, 2, D], f8, tag="m8", name="m8_0")]
            nc.sync.dma_start(mnext[0][:], mh_t[0])
            nc.sync.dma_start(mnext[1][:], m8_t[0])

            for h in range(n_heads):
                mhi, m8h = mnext
                w2h = whead.tile([P, ED, D], f16, tag="w2")
                nc.sync.dma_start(w2h[:], w2_t[h])

                # ---- u proj: uT[e,s] = (sum_d 32M[d,e] xT[d,s] + 32r[e])/32
                # fp16 hi*hi pass + one DoubleRow fp8 pass for (lo*full +
                # hi*lo); epilogue writes fp16 hi + fp8 (lo, hi) operand set
                # for the score matmul.
                uh = work.tile([P, ED, S], f16, tag="uh", bufs=1)
                u8 = work.tile([P, ED, 2, S], f8, tag="u8", bufs=1)
                for et in range(ED):
                    e_sl = slice(et * P, (et + 1) * P)
                    for sc_ in range(2):
                        s_sl = slice(sc_ * 512, (sc_ + 1) * 512)
                        ps = mmps.tile([P, 512], f32, tag="mm512")
                        # interleave fp16 and DoubleRow per k-chunk so each
                        # DR LDWEIGHTS (256 cols, no FWL) hides under the
                        # preceding 213ns fp16 matmul
                        for dt_ in range(ED):
                            nc.tensor.matmul(
                                ps[:], mhi[:, dt_, e_sl], xh[:, dt_, s_sl],
                                start=(dt_ == 0), stop=False)
                            nc.tensor.matmul(
                                ps[:], m8h[:, dt_, :, e_sl],
                                x8[:, dt_, :, s_sl],
                                start=False, stop=(dt_ == ED - 1),
                                perf_mode=DR)
                        # ps = (ps + 32r)/32, then split: fp16 hi + fp8 pair
                        nc.vector.tensor_scalar(
                            ps[:], ps[:], rsb[:, h, et:et + 1], 1.0 / MSCALE,
                            op0=ALU.add, op1=ALU.mult)
                        nc.scalar.activation(uh[:, et, s_sl], ps[:], AF.Copy)
                        nc.scalar.activation(
                            u8[:, et, 1, s_sl], ps[:], AF.Copy)
                        nc.vector.tensor_sub(
                            u8[:, et, 0, s_sl], ps[:], uh[:, et, s_sl])

                # prefetch next head's M during this head's compute
                if h + 1 < n_heads:
                    mnext = [
                        whead.tile([P, ED, D], f16, tag="mh",
                                   name=f"mhi{h + 1}"),
                        whead.tile([P, ED, 2, D], f8, tag="m8",
                                   name=f"m8_{h + 1}")]
                    nc.sync.dma_start(mnext[0][:], mh_t[h + 1])
                    nc.sync.dma_start(mnext[1][:], m8_t[h + 1])

                # ---- w projection (1-pass): w[t,n] = sum_d xT[d,t] W2[d,n]
                wsb = work.tile([P, SD, D], f16, tag="w", bufs=1)
                for tt in range(SD):
                    t_sl = slice(tt * P, (tt + 1) * P)
                    for (n0, n1) in ((0, 512), (512, 768)):
                        ps = mmps.tile([P, 512], f32, tag="mm512")
                        for dt_ in range(ED):
                            nc.tensor.matmul(
                                ps[:, :n1 - n0], xh[:, dt_, t_sl],
                                w2h[:, dt_, n0:n1],
                                start=(dt_ == 0), stop=(dt_ == ED - 1))
                        nc.scalar.activation(
                            wsb[:, tt, n0:n1], ps[:, :n1 - n0], AF.Copy)

                # ---- scores + softmax; transposes one s-tile behind ----
                pT = work.tile([P, SD, S], f16, tag="pT", bufs=1)

                def emit_transposes(st, ptile):
                    s_sl = slice(st * P, (st + 1) * P)
                    for g in range(2):
                        tp_ps = tpps.tile([P, 4, P], f16, tag="tp")
                        for k in range(4):
                            tt = g * 4 + k
                            nc.tensor.transpose(
                                tp_ps[:, k, :], ptile[:, tt * P:(tt + 1) * P],
                                ident[:])
                        nc.vector.tensor_copy(
                            pT[:, g * 4:(g + 1) * 4, s_sl], tp_ps[:])

                pending = []
                for st in range(SD):
                    s_sl = slice(st * P, (st + 1) * P)
                    sc_ps = scps.tile([P, S], f32, tag="sc")
                    for tch in range(2):
                        t_sl = slice(tch * 512, (tch + 1) * 512)
                        for et in range(ED):
                            nc.tensor.matmul(
                                sc_ps[:, t_sl], uh[:, et, s_sl],
                                xh[:, et, t_sl],
                                start=(et == 0), stop=False)
                            nc.tensor.matmul(
                                sc_ps[:, t_sl], u8[:, et, :, s_sl],
                                x8[:, et, :, t_sl],
                                start=False, stop=(et == ED - 1),
                                perf_mode=DR)
                    negmax = small.tile([P, 1], f32, tag="negmax")
                    nc.vector.tensor_reduce(
                        negmax[:], sc_ps[:], axis=mybir.AxisListType.X,
                        op=mybir.AluOpType.max, negate=True)
                    bias8 = small.tile([P, 1], f32, tag="bias8")
                    nc.vector.tensor_scalar_mul(bias8[:], negmax[:], SCALE)
                    ptile = work.tile([P, S], f16, tag="p")
                    sumexp = small.tile([P, 1], f32, tag="sumexp")
                    nc.scalar.activation(
                        ptile[:], sc_ps[:], AF.Exp,
                        bias=bias8[:], scale=SCALE, accum_out=sumexp[:])
                    recip = small.tile([P, 1], f32, tag="recip")
                    nc.vector.reciprocal(recip[:], sumexp[:])
                    nc.vector.tensor_scalar_mul(ptile[:], ptile[:], recip[:])
                    pending.append((st, ptile))
                    if len(pending) == 2:
                        emit_transposes(*pending.pop(0))
                emit_transposes(*pending.pop(0))

                # ---- out[s,n] += sum_t P[s,t] w[t,n], accumulated over heads
                for st in range(SD):
                    s_sl = slice(st * P, (st + 1) * P)
                    for (n0, n1) in ((0, 512), (512, 768)):
                        pr = mmps.tile([P, 512], f32, tag="mm512")
                        for tt in range(SD):
                            nc.tensor.matmul(
                                pr[:, :n1 - n0], pT[:, tt, s_sl],
                                wsb[:, tt, n0:n1],
                                start=(tt == 0), stop=(tt == SD - 1))
                        if h == 0:
                            nc.vector.tensor_copy(
                                acc[:, st, n0:n1], pr[:, :n1 - n0])
                        else:
                            nc.vector.tensor_add(
                                out=acc[:, st, n0:n1], in0=acc[:, st, n0:n1],
                                in1=pr[:, :n1 - n0])

            for st in range(SD):
                nc.sync.dma_start(out_t[:, st, :], acc[:, st, :])

    nc.compile()
    return nc


def _get_nc():
    if "nc" not in _CACHE:
        _CACHE["nc"] = _build_nc()
    return _CACHE["nc"]


def _prepare(x, Wq, bq, Wk, bk, Wv, bv, Wp, bp):
    f16 = np.float16
    e5 = ml_dtypes.float8_e5m2
    x = np.asarray(x, dtype=np.float32)
    Wq = np.asarray(Wq, dtype=np.float32)
    Wk = np.asarray(Wk, dtype=np.float32)
    Wv = np.asarray(Wv, dtype=np.float32)
    Wp = np.asarray(Wp, dtype=np.float32)
    bq = np.asarray(bq, dtype=np.float32)
    bv = np.asarray(bv, dtype=np.float32)
    bp = np.asarray(bp, dtype=np.float32)

    # scores = x M x^T + ones (x r)^T with M = Wq Wk^T, r = Wk bq.
    # (x Wq bk^T and bq.bk shift rows uniformly and cancel in softmax.)
    M = np.matmul(Wq, Wk.transpose(0, 2, 1))          # [H, D, D]
    r = np.matmul(Wk, bq[:, :, None])[:, :, 0]        # [H, D]
    wp3 = Wp.reshape(H, D, D)
    W2 = np.matmul(Wv, wp3)                           # [H, D, D]

    # bv contributes sum_h bv_h @ Wp_h to every output row (softmax rows sum
    # to 1); fold it and bp into one host-side bias.  bk dropped entirely.
    bp_eff = (bp.astype(np.float64)
              + np.einsum('hd,hde->e', bv.astype(np.float64),
                          wp3.astype(np.float64))).astype(np.float32)

    M32 = MSCALE * M
    m_hi = M32.astype(f16)
    m8 = np.empty((H, D, 2, D), dtype=e5)
    m8[:, :, 0, :] = (M32 - m_hi.astype(np.float32)).astype(e5)  # lo (x full)
    m8[:, :, 1, :] = M32.astype(e5)                              # hi (x lo)

    shared = {
        "m_hi": m_hi, "m8": m8,
        "w2": W2.astype(f16),
        "r": (MSCALE * r).astype(np.float32),
    }
    in_maps = []
    for b in range(B):
        xT = np.ascontiguousarray(x[b].T)
        xt_hi = xT.astype(f16)
        x8 = np.empty((D, 2, S), dtype=e5)
        x8[:, 0, :] = xT.astype(e5)                              # full
        x8[:, 1, :] = (xT - xt_hi.astype(np.float32)).astype(e5)  # lo
        in_maps.append({"xT_hi": xt_hi, "x8": x8, **shared})
    return in_maps, bp_eff


def kernel(x, Wq, bq, Wk, bk, Wv, bv, Wp, bp):
    from concourse.bass_utils import run_bass_kernel_spmd

    in_maps, bp_eff = _prepare(x, Wq, bq, Wk, bk, Wv, bv, Wp, bp)
    nc = _get_nc()
    res = run_bass_kernel_spmd(nc, in_maps, list(range(B)))
    out = np.stack([res.results[b]["out"] for b in range(B)], axis=0)
    out = out + bp_eff[None, None, :]
    return out.astype(np.float32)


# revision 21
# speedup vs baseline: 2.2509x; 1.0004x over previous
"""Trainium2 Bass kernel for nn_MultiHeadAttention_65352222376626.

Reference computation (B=8, S=1024, D=768, H=12):
    q = einsum('bsd,hde->bhse', x, Wq) + bq      # per-head full-width projections
    k, v likewise
    scores = einsum('bhse,bhte->bhst', q, k) * sqrt(64)
    attn = softmax(scores, -1)
    o = einsum('bhst,bhte->bhse', attn, v)
    out = concat_heads(o) @ Wp + bp

Sharding: pure batch-parallel - B == n_cores == 8, one batch element per
NeuronCore, full weights replicated per core.  No collectives needed.

Algebraic restructure: since softmax is row-shift invariant,
    scores = x (Wq Wk^T) x^T + ones_s (x Wk bq)^T   [+ row-const terms dropped]
so we precompute M_h = Wq_h @ Wk_h^T and r_h = Wk_h @ bq_h on the host and
replace the q-proj + k-proj + scores pipeline (3 big matmuls) with
u = x@M + r followed by scores = u @ x^T (2 big matmuls).  Likewise by
associativity (P@v)@Wp = P@(x@(Wv@Wp)), so W2_h = Wv_h @ Wp_h is
precomputed and the v-proj + attn@v + out-proj trio becomes
w = x@W2 then out += P@w (2 big matmuls).

Numerics: the softmax is near-argmax (score std ~222, top-2 gaps ~60), so
logit errors flip argmaxes and blow up the absmax metric; the u-proj and
score matmul operands need ~16+ mantissa bits.  Each runs as one fp16
hi*hi pass plus ONE fp8-e5m2 DoubleRow matmul that computes both
correction terms (lo*full + hi*lo) as a K-interleaved pair at 2x rate -
1.5 effective passes instead of 3.  The correction terms are ~2^-11 of
the main term, so 3-bit e5m2 mantissas suffice (validated: absmax rel
err 2.3e-3 vs fp32 reference).  M is pre-scaled by 32 on the host so its
fp8 splits stay in e5m2's normal range; the 1/32 is folded into the
PSUM->SBUF epilogue.  The w/out path is tolerance-insensitive and runs
single-pass fp16.  bk shifts score rows by a constant and cancels in
softmax; bv's contribution is sum_h bv_h @ Wp_h (softmax rows sum to 1),
folded with bp into one host-side bias add.
"""

import numpy as np
import ml_dtypes

B, S, D, H = 8, 1024, 768, 12
P = 128
SD = S // P   # 8 tiles along the sequence axis
ED = D // P   # 6 tiles along the feature axis
SCALE = 8.0   # sqrt(head_dim=64); reference multiplies scores by this
MSCALE = 32.0  # host pre-scale of M so fp8 splits stay in e5m2 range

_CACHE = {}


def _build_nc(n_heads=H):
    import concourse.tile as tile
    from concourse import bacc, mybir
    from concourse.masks import make_identity

    f32 = mybir.dt.float32
    f16 = mybir.dt.float16
    f8 = mybir.dt.float8e5
    AF = mybir.ActivationFunctionType
    ALU = mybir.AluOpType
    DR = mybir.MatmulPerfMode.DoubleRow

    nc = bacc.Bacc()

    # ---- DRAM I/O (per core) ----
    xh_d = nc.dram_tensor("xT_hi", [D, S], f16, kind="ExternalInput")
    x8_d = nc.dram_tensor("x8", [D, 2, S], f8, kind="ExternalInput")
    mh_d = nc.dram_tensor("m_hi", [H, D, D], f16, kind="ExternalInput")
    m8_d = nc.dram_tensor("m8", [H, D, 2, D], f8, kind="ExternalInput")
    w2_d = nc.dram_tensor("w2", [H, D, D], f16, kind="ExternalInput")
    r_d = nc.dram_tensor("r", [H, D], f32, kind="ExternalInput")
    out_d = nc.dram_tensor("out", [S, D], f32, kind="ExternalOutput")

    # partition-tiled DRAM views
    xh_t = xh_d.rearrange("(o p) s -> p o s", p=P)          # [128, ED, S]
    x8_t = x8_d.rearrange("(o p) t s -> p o t s", p=P)      # [128, ED, 2, S]
    mh_t = mh_d.rearrange("h (o p) e -> h p o e", p=P)      # [H, 128, ED, D]
    m8_t = m8_d.rearrange("h (o p) t e -> h p o t e", p=P)  # [H, 128, ED, 2, D]
    w2_t = w2_d.rearrange("h (o p) e -> h p o e", p=P)
    r_t = r_d.rearrange("h (o p) -> p h o", p=P)            # [128, H, ED]
    out_t = out_d.rearrange("(o p) d -> p o d", p=P)        # [128, SD, D]

    with tile.TileContext(nc) as tc:
        with (
            tc.tile_pool(name="persist", bufs=1) as persist,
            tc.tile_pool(name="whead", bufs=2) as whead,
            tc.tile_pool(name="work", bufs=2) as work,
            tc.tile_pool(name="small", bufs=4) as small,
            tc.tile_pool(name="mmps", bufs=2, space="PSUM") as mmps,
            tc.tile_pool(name="tpps", bufs=2, space="PSUM") as tpps,
            tc.tile_pool(name="scps", bufs=2, space="PSUM") as scps,
        ):
            # ---- persistent tiles ----
            xh = persist.tile([P, ED, S], f16)
            nc.sync.dma_start(xh[:], xh_t)
            x8 = persist.tile([P, ED, 2, S], f8)
            nc.sync.dma_start(x8[:], x8_t)
            rsb = persist.tile([P, H, ED], f32)
            nc.sync.dma_start(rsb[:], r_t)
            ident = persist.tile([P, P], f16)
            make_identity(nc, ident)
            acc = persist.tile([P, SD, D], f32)     # final accumulator

            w2first = whead.tile([P, ED, D], f16, tag="w2", name="w2_0")
            nc.sync.dma_start(w2first[:], w2_t[0])
            mnext = [whead.tile([P, ED, D], f16, tag="mh", name="mhi0"),
                     whead.tile([P, ED, 2, D], f8, tag="m8", name="m8_0")]
            nc.sync.dma_start(mnext[0][:], mh_t[0])
            nc.sync.dma_start(mnext[1][:], m8_t[0])

            for h in range(n_heads):
                mhi, m8h = mnext
                if h == 0:
                    w2h = w2first
                else:
                    w2h = whead.tile([P, ED, D], f16, tag="w2")
                    nc.sync.dma_start(w2h[:], w2_t[h])

                def emit_uproj(h, mhi, m8h, uh, u8):
                    # u[e,s] = (sum_d 32M[d,e] xT[d,s] + 32r[e])/32
                    # fp16 hi*hi pass + one DoubleRow fp8 pass for (lo*full
                    # + hi*lo); epilogue writes fp16 hi + fp8 (lo, hi)
                    # operand set for the score matmul.
                    for et in range(ED):
                        e_sl = slice(et * P, (et + 1) * P)
                        for sc_ in range(2):
                            s_sl = slice(sc_ * 512, (sc_ + 1) * 512)
                            ps = mmps.tile([P, 512], f32, tag="mm512")
                            for dt_ in range(ED):
                                nc.tensor.matmul(
                                    ps[:], mhi[:, dt_, e_sl],
                                    xh[:, dt_, s_sl],
                                    start=(dt_ == 0), stop=False)
                                nc.tensor.matmul(
                                    ps[:], m8h[:, dt_, :, e_sl],
                                    x8[:, dt_, :, s_sl],
                                    start=False, stop=(dt_ == ED - 1),
                                    perf_mode=DR)
                            # ps = (ps + 32r)/32, then fp16 hi + fp8 pair
                            nc.vector.tensor_scalar(
                                ps[:], ps[:], rsb[:, h, et:et + 1],
                                1.0 / MSCALE, op0=ALU.add, op1=ALU.mult)
                            nc.scalar.activation(
                                uh[:, et, s_sl], ps[:], AF.Copy)
                            nc.scalar.activation(
                                u8[:, et, 1, s_sl], ps[:], AF.Copy)
                            nc.vector.tensor_sub(
                                u8[:, et, 0, s_sl], ps[:], uh[:, et, s_sl])

                def emit_wproj(w2h, wsb):
                    # w[t,n] = sum_d xT[d,t] W2[d,n], single-pass fp16
                    for tt in range(SD):
                        t_sl = slice(tt * P, (tt + 1) * P)
                        for (n0, n1) in ((0, 512), (512, 768)):
                            ps = mmps.tile([P, 512], f32, tag="mm512")
                            for dt_ in range(ED):
                                nc.tensor.matmul(
                                    ps[:, :n1 - n0], xh[:, dt_, t_sl],
                                    w2h[:, dt_, n0:n1],
                                    start=(dt_ == 0), stop=(dt_ == ED - 1))
                            nc.scalar.activation(
                                wsb[:, tt, n0:n1], ps[:, :n1 - n0], AF.Copy)

                uh = work.tile([P, ED, S], f16, tag="uh", bufs=1)
                u8 = work.tile([P, ED, 2, S], f8, tag="u8", bufs=1)
                wsb = work.tile([P, SD, D], f16, tag="w", bufs=1)
                if h == 0:
                    # w-proj first: it only needs xh + w2 (2.7MB of DMA) so
                    # the PE starts while x8/m_hi/m8 are still streaming in
                    emit_wproj(w2h, wsb)
                    emit_uproj(h, mhi, m8h, uh, u8)
                else:
                    emit_uproj(h, mhi, m8h, uh, u8)

                # prefetch next head's M during this head's compute
                if h + 1 < n_heads:
                    mnext = [
                        whead.tile([P, ED, D], f16, tag="mh",
                                   name=f"mhi{h + 1}"),
                        whead.tile([P, ED, 2, D], f8, tag="m8",
                                   name=f"m8_{h + 1}")]
                    nc.sync.dma_start(mnext[0][:], mh_t[h + 1])
                    nc.sync.dma_start(mnext[1][:], m8_t[h + 1])

                if h > 0:
                    emit_wproj(w2h, wsb)

                # ---- scores + softmax; transposes one s-tile behind ----
                pT = work.tile([P, SD, S], f16, tag="pT", bufs=1)

                def emit_transposes(st, ptile):
                    s_sl = slice(st * P, (st + 1) * P)
                    for g in range(2):
                        tp_ps = tpps.tile([P, 4, P], f16, tag="tp")
                        for k in range(4):
                            tt = g * 4 + k
                            nc.tensor.transpose(
                                tp_ps[:, k, :], ptile[:, tt * P:(tt + 1) * P],
                                ident[:])
                        nc.vector.tensor_copy(
                            pT[:, g * 4:(g + 1) * 4, s_sl], tp_ps[:])

                pending = []
                for st in range(SD):
                    s_sl = slice(st * P, (st + 1) * P)
                    sc_ps = scps.tile([P, S], f32, tag="sc")
                    for tch in range(2):
                        t_sl = slice(tch * 512, (tch + 1) * 512)
                        for et in range(ED):
                            nc.tensor.matmul(
                                sc_ps[:, t_sl], uh[:, et, s_sl],
                                xh[:, et, t_sl],
                                start=(et == 0), stop=False)
                            nc.tensor.matmul(
                                sc_ps[:, t_sl], u8[:, et, :, s_sl],
                                x8[:, et, :, t_sl],
                                start=False, stop=(et == ED - 1),
                                perf_mode=DR)
                    negmax = small.tile([P, 1], f32, tag="negmax")
                    nc.vector.tensor_reduce(
                        negmax[:], sc_ps[:], axis=mybir.AxisListType.X,
                        op=mybir.AluOpType.max, negate=True)
                    bias8 = small.tile([P, 1], f32, tag="bias8")
                    nc.vector.tensor_scalar_mul(bias8[:], negmax[:], SCALE)
                    ptile = work.tile([P, S], f16, tag="p")
                    sumexp = small.tile([P, 1], f32, tag="sumexp")
                    nc.scalar.activation(
                        ptile[:], sc_ps[:], AF.Exp,
                        bias=bias8[:], scale=SCALE, accum_out=sumexp[:])
                    recip = small.tile([P, 1], f32, tag="recip")
                    nc.vector.reciprocal(recip[:], sumexp[:])
                    nc.vector.tensor_scalar_mul(ptile[:], ptile[:], recip[:])
                    pending.append((st, ptile))
                    if len(pending) == 2:
                        emit_transposes(*pending.pop(0))

                # ---- out[s,n] += sum_t P[s,t] w[t,n], accumulated over heads
                # The last s-tile's transposes are emitted after the first
                # out-group's tt=0..6 matmuls so the PE FIFO isn't blocked
                # behind softmax(st=7) latency (tt=7 is the only dependent).
                last_tp = pending.pop(0)
                for st in range(SD):
                    s_sl = slice(st * P, (st + 1) * P)
                    for (n0, n1) in ((0, 512), (512, 768)):
                        pr = mmps.tile([P, 512], f32, tag="mm512")
                        for tt in range(SD):
                            if last_tp is not None and tt == SD - 1:
                                emit_transposes(*last_tp)
                                last_tp = None
                            nc.tensor.matmul(
                                pr[:, :n1 - n0], pT[:, tt, s_sl],
                                wsb[:, tt, n0:n1],
                                start=(tt == 0), stop=(tt == SD - 1))
                        if h == 0:
                            nc.vector.tensor_copy(
                                acc[:, st, n0:n1], pr[:, :n1 - n0])
                        else:
                            nc.vector.tensor_add(
                                out=acc[:, st, n0:n1], in0=acc[:, st, n0:n1],
                                in1=pr[:, :n1 - n0])
                            if h == n_heads - 1 and n0 == 512:
                                # stream the finished s-tile out during the
                                # last head's remaining compute
                                nc.sync.dma_start(
                                    out_t[:, st, :], acc[:, st, :])

    nc.compile()
    return nc


def _get_nc():
    if "nc" not in _CACHE:
        _CACHE["nc"] = _build_nc()
    return _CACHE["nc"]


def _prepare(x, Wq, bq, Wk, bk, Wv, bv, Wp, bp):
    f16 = np.float16
    e5 = ml_dtypes.float8_e5m2
    x = np.asarray(x, dtype=np.float32)
    Wq = np.asarray(Wq, dtype=np.float32)
    Wk = np.asarray(Wk, dtype=np.float32)
    Wv = np.asarray(Wv, dtype=np.float32)
    Wp = np.asarray(Wp, dtype=np.float32)
    bq = np.asarray(bq, dtype=np.float32)
    bv = np.asarray(bv, dtype=np.float32)
    bp = np.asarray(bp, dtype=np.float32)

    # scores = x M x^T + ones (x r)^T with M = Wq Wk^T, r = Wk bq.
    # (x Wq bk^T and bq.bk shift rows uniformly and cancel in softmax.)
    M = np.matmul(Wq, Wk.transpose(0, 2, 1))          # [H, D, D]
    r = np.matmul(Wk, bq[:, :, None])[:, :, 0]        # [H, D]
    wp3 = Wp.reshape(H, D, D)
    W2 = np.matmul(Wv, wp3)                           # [H, D, D]

    # bv contributes sum_h bv_h @ Wp_h to every output row (softmax rows sum
    # to 1); fold it and bp into one host-side bias.  bk dropped entirely.
    bp_eff = (bp.astype(np.float64)
              + np.einsum('hd,hde->e', bv.astype(np.float64),
                          wp3.astype(np.float64))).astype(np.float32)

    M32 = MSCALE * M
    m_hi = M32.astype(f16)
    m8 = np.empty((H, D, 2, D), dtype=e5)
    m8[:, :, 0, :] = (M32 - m_hi.astype(np.float32)).astype(e5)  # lo (x full)
    m8[:, :, 1, :] = M32.astype(e5)                              # hi (x lo)

    shared = {
        "m_hi": m_hi, "m8": m8,
        "w2": W2.astype(f16),
        "r": (MSCALE * r).astype(np.float32),
    }
    in_maps = []
    for b in range(B):
        xT = np.ascontiguousarray(x[b].T)
        xt_hi = xT.astype(f16)
        x8 = np.empty((D, 2, S), dtype=e5)
        x8[:, 0, :] = xT.astype(e5)                              # full
        x8[:, 1, :] = (xT - xt_hi.astype(np.float32)).astype(e5)  # lo
        in_maps.append({"xT_hi": xt_hi, "x8": x8, **shared})
    return in_maps, bp_eff


def kernel(x, Wq, bq, Wk, bk, Wv, bv, Wp, bp):
    from concourse.bass_utils import run_bass_kernel_spmd

    in_maps, bp_eff = _prepare(x, Wq, bq, Wk, bk, Wv, bv, Wp, bp)
    nc = _get_nc()
    res = run_bass_kernel_spmd(nc, in_maps, list(range(B)))
    out = np.stack([res.results[b]["out"] for b in range(B)], axis=0)
    out = out + bp_eff[None, None, :]
    return out.astype(np.float32)


# revision 23
# speedup vs baseline: 2.2548x; 1.0018x over previous
"""Trainium2 Bass kernel for nn_MultiHeadAttention_65352222376626.

Reference computation (B=8, S=1024, D=768, H=12):
    q = einsum('bsd,hde->bhse', x, Wq) + bq      # per-head full-width projections
    k, v likewise
    scores = einsum('bhse,bhte->bhst', q, k) * sqrt(64)
    attn = softmax(scores, -1)
    o = einsum('bhst,bhte->bhse', attn, v)
    out = concat_heads(o) @ Wp + bp

Sharding: pure batch-parallel - B == n_cores == 8, one batch element per
NeuronCore, full weights replicated per core.  No collectives needed.

Algebraic restructure: since softmax is row-shift invariant,
    scores = x (Wq Wk^T) x^T + ones_s (x Wk bq)^T   [+ row-const terms dropped]
so we precompute M_h = Wq_h @ Wk_h^T and r_h = Wk_h @ bq_h on the host and
replace the q-proj + k-proj + scores pipeline (3 big matmuls) with
u = x@M + r followed by scores = u @ x^T (2 big matmuls).  Likewise by
associativity (P@v)@Wp = P@(x@(Wv@Wp)), so W2_h = Wv_h @ Wp_h is
precomputed and the v-proj + attn@v + out-proj trio becomes
w = x@W2 then out += P@w (2 big matmuls).

Numerics: the softmax is near-argmax (score std ~222, top-2 gaps ~60), so
logit errors flip argmaxes and blow up the absmax metric; the u-proj and
score matmul operands need ~16+ mantissa bits.  Each runs as one fp16
hi*hi pass plus ONE fp8-e5m2 DoubleRow matmul that computes both
correction terms (lo*full + hi*lo) as a K-interleaved pair at 2x rate -
1.5 effective passes instead of 3.  The correction terms are ~2^-11 of
the main term, so 3-bit e5m2 mantissas suffice (validated: absmax rel
err 2.3e-3 vs fp32 reference).  M is pre-scaled by 32 on the host so its
fp8 splits stay in e5m2's normal range; the 1/32 is folded into the
PSUM->SBUF epilogue.  The w/out path is tolerance-insensitive and runs
single-pass fp16.  bk shifts score rows by a constant and cancels in
softmax; bv's contribution is sum_h bv_h @ Wp_h (softmax rows sum to 1),
folded with bp into one host-side bias add.
"""

import numpy as np
import ml_dtypes

B, S, D, H = 8, 1024, 768, 12
P = 128
SD = S // P   # 8 tiles along the sequence axis
ED = D // P   # 6 tiles along the feature axis
SCALE = 8.0   # sqrt(head_dim=64); reference multiplies scores by this
MSCALE = 32.0  # host pre-scale of M so fp8 splits stay in e5m2 range

_CACHE = {}


def _build_nc(n_heads=H):
    import concourse.tile as tile
    from concourse import bacc, mybir
    from concourse.masks import make_identity

    f32 = mybir.dt.float32
    f16 = mybir.dt.float16
    f8 = mybir.dt.float8e5
    AF = mybir.ActivationFunctionType
    ALU = mybir.AluOpType
    DR = mybir.MatmulPerfMode.DoubleRow

    nc = bacc.Bacc()

    # ---- DRAM I/O (per core) ----
    xh_d = nc.dram_tensor("xT_hi", [D, S], f16, kind="ExternalInput")
    x8_d = nc.dram_tensor("x8", [D, 2, S], f8, kind="ExternalInput")
    mh_d = nc.dram_tensor("m_hi", [H, D, D], f16, kind="ExternalInput")
    m8_d = nc.dram_tensor("m8", [H, D, 2, D], f8, kind="ExternalInput")
    w2_d = nc.dram_tensor("w2", [H, D, D], f16, kind="ExternalInput")
    r_d = nc.dram_tensor("r", [H, D], f32, kind="ExternalInput")
    out_d = nc.dram_tensor("out", [S, D], f32, kind="ExternalOutput")

    # partition-tiled DRAM views
    xh_t = xh_d.rearrange("(o p) s -> p o s", p=P)          # [128, ED, S]
    x8_t = x8_d.rearrange("(o p) t s -> p o t s", p=P)      # [128, ED, 2, S]
    mh_t = mh_d.rearrange("h (o p) e -> h p o e", p=P)      # [H, 128, ED, D]
    m8_t = m8_d.rearrange("h (o p) t e -> h p o t e", p=P)  # [H, 128, ED, 2, D]
    w2_t = w2_d.rearrange("h (o p) e -> h p o e", p=P)
    r_t = r_d.rearrange("h (o p) -> p h o", p=P)            # [128, H, ED]
    out_t = out_d.rearrange("(o p) d -> p o d", p=P)        # [128, SD, D]

    with tile.TileContext(nc) as tc:
        with (
            tc.tile_pool(name="persist", bufs=1) as persist,
            tc.tile_pool(name="whead", bufs=2) as whead,
            tc.tile_pool(name="work", bufs=2) as work,
            tc.tile_pool(name="small", bufs=4) as small,
            tc.tile_pool(name="mmps", bufs=2, space="PSUM") as mmps,
            tc.tile_pool(name="tpps", bufs=2, space="PSUM") as tpps,
            tc.tile_pool(name="scps", bufs=2, space="PSUM") as scps,
        ):
            # ---- persistent tiles ----
            # startup critical path: head 0 runs w-proj first, which needs
            # only xh + w2 -- put those two on the sync DMA queue and
            # everything else on the scalar (Activation) HWDGE queue so
            # they stream concurrently
            xh = persist.tile([P, ED, S], f16)
            nc.sync.dma_start(xh[:], xh_t)
            w2first = whead.tile([P, ED, D], f16, tag="w2", name="w2_0")
            nc.sync.dma_start(w2first[:], w2_t[0])
            x8 = persist.tile([P, ED, 2, S], f8)
            nc.scalar.dma_start(x8[:], x8_t)
            rsb = persist.tile([P, H, ED], f32)
            nc.scalar.dma_start(rsb[:], r_t)
            ident = persist.tile([P, P], f16)
            make_identity(nc, ident)
            acc = persist.tile([P, SD, D], f32)     # final accumulator

            mnext = [whead.tile([P, ED, D], f16, tag="mh", name="mhi0"),
                     whead.tile([P, ED, 2, D], f8, tag="m8", name="m8_0")]
            nc.scalar.dma_start(mnext[0][:], mh_t[0])
            nc.scalar.dma_start(mnext[1][:], m8_t[0])

            for h in range(n_heads):
                mhi, m8h = mnext
                if h == 0:
                    w2h = w2first
                else:
                    w2h = whead.tile([P, ED, D], f16, tag="w2")
                    nc.sync.dma_start(w2h[:], w2_t[h])

                def emit_uproj(h, mhi, m8h, uh, u8):
                    # u[e,s] = (sum_d 32M[d,e] xT[d,s] + 32r[e])/32
                    # fp16 hi*hi pass + one DoubleRow fp8 pass for (lo*full
                    # + hi*lo); epilogue writes fp16 hi + fp8 (lo, hi)
                    # operand set for the score matmul.
                    for et in range(ED):
                        e_sl = slice(et * P, (et + 1) * P)
                        for sc_ in range(2):
                            s_sl = slice(sc_ * 512, (sc_ + 1) * 512)
                            if et == 0 and sc_ == 0:
                                # first group borrows a score-pool bank (idle
                                # at head start) so it needn't wait for the
                                # previous head's out-accum epilogue
                                ps = scps.tile(
                                    [P, S], f32, tag="sc",
                                    name="ups0")[:, :512]
                            else:
                                ps = mmps.tile([P, 512], f32, tag="mm512")
                            for dt_ in range(ED):
                                nc.tensor.matmul(
                                    ps[:], mhi[:, dt_, e_sl],
                                    xh[:, dt_, s_sl],
                                    start=(dt_ == 0), stop=False)
                                nc.tensor.matmul(
                                    ps[:], m8h[:, dt_, :, e_sl],
                                    x8[:, dt_, :, s_sl],
                                    start=False, stop=(dt_ == ED - 1),
                                    perf_mode=DR)
                            # ps = (ps + 32r)/32, then fp16 hi + fp8 pair
                            nc.vector.tensor_scalar(
                                ps[:], ps[:], rsb[:, h, et:et + 1],
                                1.0 / MSCALE, op0=ALU.add, op1=ALU.mult)
                            nc.scalar.activation(
                                uh[:, et, s_sl], ps[:], AF.Copy)
                            nc.scalar.activation(
                                u8[:, et, 1, s_sl], ps[:], AF.Copy)
                            nc.vector.tensor_sub(
                                u8[:, et, 0, s_sl], ps[:], uh[:, et, s_sl])

                def emit_wproj(w2h, wsb):
                    # w[t,n] = sum_d xT[d,t] W2[d,n], single-pass fp16
                    for tt in range(SD):
                        t_sl = slice(tt * P, (tt + 1) * P)
                        for (n0, n1) in ((0, 512), (512, 768)):
                            ps = mmps.tile([P, 512], f32, tag="mm512")
                            for dt_ in range(ED):
                                nc.tensor.matmul(
                                    ps[:, :n1 - n0], xh[:, dt_, t_sl],
                                    w2h[:, dt_, n0:n1],
                                    start=(dt_ == 0), stop=(dt_ == ED - 1))
                            nc.scalar.activation(
                                wsb[:, tt, n0:n1], ps[:, :n1 - n0], AF.Copy)

                uh = work.tile([P, ED, S], f16, tag="uh", bufs=1)
                u8 = work.tile([P, ED, 2, S], f8, tag="u8", bufs=1)
                wsb = work.tile([P, SD, D], f16, tag="w", bufs=1)
                if h == 0:
                    # w-proj first: it only needs xh + w2 (2.7MB of DMA) so
                    # the PE starts while x8/m_hi/m8 are still streaming in
                    emit_wproj(w2h, wsb)
                    emit_uproj(h, mhi, m8h, uh, u8)
                else:
                    emit_uproj(h, mhi, m8h, uh, u8)

                # prefetch next head's M during this head's compute
                if h + 1 < n_heads:
                    mnext = [
                        whead.tile([P, ED, D], f16, tag="mh",
                                   name=f"mhi{h + 1}"),
                        whead.tile([P, ED, 2, D], f8, tag="m8",
                                   name=f"m8_{h + 1}")]
                    nc.sync.dma_start(mnext[0][:], mh_t[h + 1])
                    nc.sync.dma_start(mnext[1][:], m8_t[h + 1])

                if h > 0:
                    emit_wproj(w2h, wsb)

                # ---- scores + softmax; transposes one s-tile behind ----
                pT = work.tile([P, SD, S], f16, tag="pT", bufs=1)

                def emit_transposes(st, ptile):
                    s_sl = slice(st * P, (st + 1) * P)
                    for g in range(2):
                        tp_ps = tpps.tile([P, 4, P], f16, tag="tp")
                        for k in range(4):
                            tt = g * 4 + k
                            nc.tensor.transpose(
                                tp_ps[:, k, :], ptile[:, tt * P:(tt + 1) * P],
                                ident[:])
                        nc.vector.tensor_copy(
                            pT[:, g * 4:(g + 1) * 4, s_sl], tp_ps[:])

                pending = []
                for st in range(SD):
                    s_sl = slice(st * P, (st + 1) * P)
                    sc_ps = scps.tile([P, S], f32, tag="sc")
                    for tch in range(2):
                        t_sl = slice(tch * 512, (tch + 1) * 512)
                        for et in range(ED):
                            nc.tensor.matmul(
                                sc_ps[:, t_sl], uh[:, et, s_sl],
                                xh[:, et, t_sl],
                                start=(et == 0), stop=False)
                            nc.tensor.matmul(
                                sc_ps[:, t_sl], u8[:, et, :, s_sl],
                                x8[:, et, :, t_sl],
                                start=False, stop=(et == ED - 1),
                                perf_mode=DR)
                    negmax = small.tile([P, 1], f32, tag="negmax")
                    nc.vector.tensor_reduce(
                        negmax[:], sc_ps[:], axis=mybir.AxisListType.X,
                        op=mybir.AluOpType.max, negate=True)
                    bias8 = small.tile([P, 1], f32, tag="bias8")
                    nc.vector.tensor_scalar_mul(bias8[:], negmax[:], SCALE)
                    ptile = work.tile([P, S], f16, tag="p")
                    sumexp = small.tile([P, 1], f32, tag="sumexp")
                    nc.scalar.activation(
                        ptile[:], sc_ps[:], AF.Exp,
                        bias=bias8[:], scale=SCALE, accum_out=sumexp[:])
                    recip = small.tile([P, 1], f32, tag="recip")
                    nc.vector.reciprocal(recip[:], sumexp[:])
                    nc.vector.tensor_scalar_mul(ptile[:], ptile[:], recip[:])
                    pending.append((st, ptile))
                    if len(pending) == 2:
                        emit_transposes(*pending.pop(0))

                # ---- out[s,n] += sum_t P[s,t] w[t,n], accumulated over heads
                # The last s-tile's transposes are emitted after the first
                # out-group's tt=0..6 matmuls so the PE FIFO isn't blocked
                # behind softmax(st=7) latency (tt=7 is the only dependent).
                last_tp = pending.pop(0)
                for st in range(SD):
                    s_sl = slice(st * P, (st + 1) * P)
                    for (n0, n1) in ((0, 512), (512, 768)):
                        pr = mmps.tile([P, 512], f32, tag="mm512")
                        for tt in range(SD):
                            if last_tp is not None and tt == SD - 1:
                                emit_transposes(*last_tp)
                                last_tp = None
                            nc.tensor.matmul(
                                pr[:, :n1 - n0], pT[:, tt, s_sl],
                                wsb[:, tt, n0:n1],
                                start=(tt == 0), stop=(tt == SD - 1))
                        if h == 0:
                            nc.vector.tensor_copy(
                                acc[:, st, n0:n1], pr[:, :n1 - n0])
                        else:
                            nc.vector.tensor_add(
                                out=acc[:, st, n0:n1], in0=acc[:, st, n0:n1],
                                in1=pr[:, :n1 - n0])
                            if h == n_heads - 1 and n0 == 512:
                                # stream the finished s-tile out during the
                                # last head's remaining compute
                                nc.sync.dma_start(
                                    out_t[:, st, :], acc[:, st, :])

    nc.compile()
    return nc


def _get_nc():
    if "nc" not in _CACHE:
        _CACHE["nc"] = _build_nc()
    return _CACHE["nc"]


def _prepare(x, Wq, bq, Wk, bk, Wv, bv, Wp, bp):
    f16 = np.float16
    e5 = ml_dtypes.float8_e5m2
    x = np.asarray(x, dtype=np.float32)
    Wq = np.asarray(Wq, dtype=np.float32)
    Wk = np.asarray(Wk, dtype=np.float32)
    Wv = np.asarray(Wv, dtype=np.float32)
    Wp = np.asarray(Wp, dtype=np.float32)
    bq = np.asarray(bq, dtype=np.float32)
    bv = np.asarray(bv, dtype=np.float32)
    bp = np.asarray(bp, dtype=np.float32)

    # scores = x M x^T + ones (x r)^T with M = Wq Wk^T, r = Wk bq.
    # (x Wq bk^T and bq.bk shift rows uniformly and cancel in softmax.)
    M = np.matmul(Wq, Wk.transpose(0, 2, 1))          # [H, D, D]
    r = np.matmul(Wk, bq[:, :, None])[:, :, 0]        # [H, D]
    wp3 = Wp.reshape(H, D, D)
    W2 = np.matmul(Wv, wp3)                           # [H, D, D]

    # bv contributes sum_h bv_h @ Wp_h to every output row (softmax rows sum
    # to 1); fold it and bp into one host-side bias.  bk dropped entirely.
    bp_eff = (bp.astype(np.float64)
              + np.einsum('hd,hde->e', bv.astype(np.float64),
                          wp3.astype(np.float64))).astype(np.float32)

    M32 = MSCALE * M
    m_hi = M32.astype(f16)
    m8 = np.empty((H, D, 2, D), dtype=e5)
    m8[:, :, 0, :] = (M32 - m_hi.astype(np.float32)).astype(e5)  # lo (x full)
    m8[:, :, 1, :] = M32.astype(e5)                              # hi (x lo)

    shared = {
        "m_hi": m_hi, "m8": m8,
        "w2": W2.astype(f16),
        "r": (MSCALE * r).astype(np.float32),
    }
    in_maps = []
    for b in range(B):
        xT = np.ascontiguousarray(x[b].T)
        xt_hi = xT.astype(f16)
        x8 = np.empty((D, 2, S), dtype=e5)
        x8[:, 0, :] = xT.astype(e5)                              # full
        x8[:, 1, :] = (xT - xt_hi.astype(np.float32)).astype(e5)  # lo
        in_maps.append({"xT_hi": xt_hi, "x8": x8, **shared})
    return in_maps, bp_eff


def kernel(x, Wq, bq, Wk, bk, Wv, bv, Wp, bp):
    from concourse.bass_utils import run_bass_kernel_spmd

    in_maps, bp_eff = _prepare(x, Wq, bq, Wk, bk, Wv, bv, Wp, bp)
    nc = _get_nc()
    res = run_bass_kernel_spmd(nc, in_maps, list(range(B)))
    out = np.stack([res.results[b]["out"] for b in range(B)], axis=0)
    out = out + bp_eff[None, None, :]
    return out.astype(np.float32)


# revision 24
# speedup vs baseline: 2.2956x; 1.0181x over previous
"""Trainium2 Bass kernel for nn_MultiHeadAttention_65352222376626.

Reference computation (B=8, S=1024, D=768, H=12):
    q = einsum('bsd,hde->bhse', x, Wq) + bq      # per-head full-width projections
    k, v likewise
    scores = einsum('bhse,bhte->bhst', q, k) * sqrt(64)
    attn = softmax(scores, -1)
    o = einsum('bhst,bhte->bhse', attn, v)
    out = concat_heads(o) @ Wp + bp

Sharding: pure batch-parallel - B == n_cores == 8, one batch element per
NeuronCore, full weights replicated per core.  No collectives needed.

Algebraic restructure: since softmax is row-shift invariant,
    scores = x (Wq Wk^T) x^T + ones_s (x Wk bq)^T   [+ row-const terms dropped]
so we precompute M_h = Wq_h @ Wk_h^T and r_h = Wk_h @ bq_h on the host and
replace the q-proj + k-proj + scores pipeline (3 big matmuls) with
u = x@M + r followed by scores = u @ x^T (2 big matmuls).  Likewise by
associativity (P@v)@Wp = P@(x@(Wv@Wp)), so W2_h = Wv_h @ Wp_h is
precomputed and the v-proj + attn@v + out-proj trio becomes
w = x@W2 then out += P@w (2 big matmuls).

Numerics: the softmax is near-argmax (score std ~222, top-2 gaps ~60), so
logit errors flip argmaxes and blow up the absmax metric; the u-proj and
score matmul operands need ~16+ mantissa bits.  Each runs as one fp16
hi*hi pass plus ONE fp8-e5m2 DoubleRow matmul that computes both
correction terms (lo*full + hi*lo) as a K-interleaved pair at 2x rate -
1.5 effective passes instead of 3.  The correction terms are ~2^-11 of
the main term, so 3-bit e5m2 mantissas suffice (validated: absmax rel
err 2.3e-3 vs fp32 reference).  M is pre-scaled by 32 on the host so its
fp8 splits stay in e5m2's normal range; the 1/32 is folded into the
PSUM->SBUF epilogue.  The w/out path is tolerance-insensitive and runs
single-pass fp16.  bk shifts score rows by a constant and cancels in
softmax; bv's contribution is sum_h bv_h @ Wp_h (softmax rows sum to 1),
folded with bp into one host-side bias add.
"""

import numpy as np
import ml_dtypes

B, S, D, H = 8, 1024, 768, 12
P = 128
SD = S // P   # 8 tiles along the sequence axis
ED = D // P   # 6 tiles along the feature axis
SCALE = 8.0   # sqrt(head_dim=64); reference multiplies scores by this
MSCALE = 32.0  # host pre-scale of M so fp8 splits stay in e5m2 range

_CACHE = {}


def _build_nc(n_heads=H):
    import concourse.tile as tile
    from concourse import bacc, mybir
    from concourse.masks import make_identity

    f32 = mybir.dt.float32
    f16 = mybir.dt.float16
    f8 = mybir.dt.float8e5
    AF = mybir.ActivationFunctionType
    ALU = mybir.AluOpType
    DR = mybir.MatmulPerfMode.DoubleRow

    nc = bacc.Bacc()

    # ---- DRAM I/O (per core) ----
    xh_d = nc.dram_tensor("xT_hi", [D, S], f16, kind="ExternalInput")
    x8_d = nc.dram_tensor("x8", [D, 2, S], f8, kind="ExternalInput")
    mh_d = nc.dram_tensor("m_hi", [H, D, D], f16, kind="ExternalInput")
    m8_d = nc.dram_tensor("m8", [H, D, 2, D], f8, kind="ExternalInput")
    w2_d = nc.dram_tensor("w2", [H, D, D], f16, kind="ExternalInput")
    r_d = nc.dram_tensor("r", [H, D], f32, kind="ExternalInput")
    out_d = nc.dram_tensor("out", [S, D], f32, kind="ExternalOutput")

    # partition-tiled DRAM views
    xh_t = xh_d.rearrange("(o p) s -> p o s", p=P)          # [128, ED, S]
    x8_t = x8_d.rearrange("(o p) t s -> p o t s", p=P)      # [128, ED, 2, S]
    mh_t = mh_d.rearrange("h (o p) e -> h p o e", p=P)      # [H, 128, ED, D]
    m8_t = m8_d.rearrange("h (o p) t e -> h p o t e", p=P)  # [H, 128, ED, 2, D]
    w2_t = w2_d.rearrange("h (o p) e -> h p o e", p=P)
    r_t = r_d.rearrange("h (o p) -> p h o", p=P)            # [128, H, ED]
    out_t = out_d.rearrange("(o p) d -> p o d", p=P)        # [128, SD, D]

    with tile.TileContext(nc) as tc:
        with (
            tc.tile_pool(name="persist", bufs=1) as persist,
            tc.tile_pool(name="whead", bufs=2) as whead,
            tc.tile_pool(name="work", bufs=2) as work,
            tc.tile_pool(name="small", bufs=4) as small,
            tc.tile_pool(name="mmps", bufs=2, space="PSUM") as mmps,
            tc.tile_pool(name="tpps", bufs=2, space="PSUM") as tpps,
            tc.tile_pool(name="scps", bufs=2, space="PSUM") as scps,
        ):
            # ---- persistent tiles ----
            # startup critical path: head 0 runs w-proj first (needs only
            # xh + w2), so order the DMA queue xh, w2, then the u-proj
            # inputs, which stream in while the w-projection computes
            xh = persist.tile([P, ED, S], f16)
            nc.sync.dma_start(xh[:], xh_t)
            w2first = whead.tile([P, ED, D], f16, tag="w2", name="w2_0")
            nc.sync.dma_start(w2first[:], w2_t[0])
            mnext = [whead.tile([P, ED, D], f16, tag="mh", name="mhi0"),
                     whead.tile([P, ED, 2, D], f8, tag="m8", name="m8_0")]
            nc.sync.dma_start(mnext[0][:], mh_t[0])
            nc.sync.dma_start(mnext[1][:], m8_t[0])
            x8 = persist.tile([P, ED, 2, S], f8)
            nc.sync.dma_start(x8[:], x8_t)
            rsb = persist.tile([P, H, ED], f32)
            nc.sync.dma_start(rsb[:], r_t)
            ident = persist.tile([P, P], f16)
            make_identity(nc, ident)
            acc = persist.tile([P, SD, D], f32)     # final accumulator

            for h in range(n_heads):
                mhi, m8h = mnext
                if h == 0:
                    w2h = w2first
                else:
                    w2h = whead.tile([P, ED, D], f16, tag="w2")
                    nc.sync.dma_start(w2h[:], w2_t[h])

                def emit_uproj(h, mhi, m8h, uh, u8):
                    # u[e,s] = (sum_d 32M[d,e] xT[d,s] + 32r[e])/32
                    # fp16 hi*hi pass + one DoubleRow fp8 pass for (lo*full
                    # + hi*lo); epilogue writes fp16 hi + fp8 (lo, hi)
                    # operand set for the score matmul.
                    for et in range(ED):
                        e_sl = slice(et * P, (et + 1) * P)
                        for sc_ in range(2):
                            s_sl = slice(sc_ * 512, (sc_ + 1) * 512)
                            if et == 0 and sc_ == 0:
                                # first group borrows a score-pool bank (idle
                                # at head start) so it needn't wait for the
                                # previous head's out-accum epilogue
                                ps = scps.tile(
                                    [P, S], f32, tag="sc",
                                    name="ups0")[:, :512]
                            else:
                                ps = mmps.tile([P, 512], f32, tag="mm512")
                            for dt_ in range(ED):
                                nc.tensor.matmul(
                                    ps[:], mhi[:, dt_, e_sl],
                                    xh[:, dt_, s_sl],
                                    start=(dt_ == 0), stop=False)
                                nc.tensor.matmul(
                                    ps[:], m8h[:, dt_, :, e_sl],
                                    x8[:, dt_, :, s_sl],
                                    start=False, stop=(dt_ == ED - 1),
                                    perf_mode=DR)
                            # ps = (ps + 32r)/32, then fp16 hi + fp8 pair
                            nc.vector.tensor_scalar(
                                ps[:], ps[:], rsb[:, h, et:et + 1],
                                1.0 / MSCALE, op0=ALU.add, op1=ALU.mult)
                            nc.scalar.activation(
                                uh[:, et, s_sl], ps[:], AF.Copy)
                            nc.scalar.activation(
                                u8[:, et, 1, s_sl], ps[:], AF.Copy)
                            nc.vector.tensor_sub(
                                u8[:, et, 0, s_sl], ps[:], uh[:, et, s_sl])

                def emit_wproj(w2h, wsb):
                    # w[t,n] = sum_d xT[d,t] W2[d,n], single-pass fp16
                    for tt in range(SD):
                        t_sl = slice(tt * P, (tt + 1) * P)
                        for (n0, n1) in ((0, 512), (512, 768)):
                            ps = mmps.tile([P, 512], f32, tag="mm512")
                            for dt_ in range(ED):
                                nc.tensor.matmul(
                                    ps[:, :n1 - n0], xh[:, dt_, t_sl],
                                    w2h[:, dt_, n0:n1],
                                    start=(dt_ == 0), stop=(dt_ == ED - 1))
                            nc.scalar.activation(
                                wsb[:, tt, n0:n1], ps[:, :n1 - n0], AF.Copy)

                uh = work.tile([P, ED, S], f16, tag="uh", bufs=1)
                u8 = work.tile([P, ED, 2, S], f8, tag="u8", bufs=1)
                wsb = work.tile([P, SD, D], f16, tag="w", bufs=1)
                if h == 0:
                    # w-proj first: it only needs xh + w2 (2.7MB of DMA) so
                    # the PE starts while x8/m_hi/m8 are still streaming in
                    emit_wproj(w2h, wsb)
                    emit_uproj(h, mhi, m8h, uh, u8)
                else:
                    emit_uproj(h, mhi, m8h, uh, u8)

                # prefetch next head's M during this head's compute
                if h + 1 < n_heads:
                    mnext = [
                        whead.tile([P, ED, D], f16, tag="mh",
                                   name=f"mhi{h + 1}"),
                        whead.tile([P, ED, 2, D], f8, tag="m8",
                                   name=f"m8_{h + 1}")]
                    nc.sync.dma_start(mnext[0][:], mh_t[h + 1])
                    nc.sync.dma_start(mnext[1][:], m8_t[h + 1])

                if h > 0:
                    emit_wproj(w2h, wsb)

                # ---- scores + softmax; transposes one s-tile behind ----
                pT = work.tile([P, SD, S], f16, tag="pT", bufs=1)

                def emit_transposes(st, ptile):
                    s_sl = slice(st * P, (st + 1) * P)
                    for g in range(2):
                        tp_ps = tpps.tile([P, 4, P], f16, tag="tp")
                        for k in range(4):
                            tt = g * 4 + k
                            nc.tensor.transpose(
                                tp_ps[:, k, :], ptile[:, tt * P:(tt + 1) * P],
                                ident[:])
                        nc.vector.tensor_copy(
                            pT[:, g * 4:(g + 1) * 4, s_sl], tp_ps[:])

                pending = []
                for st in range(SD):
                    s_sl = slice(st * P, (st + 1) * P)
                    sc_ps = scps.tile([P, S], f32, tag="sc")
                    for tch in range(2):
                        t_sl = slice(tch * 512, (tch + 1) * 512)
                        for et in range(ED):
                            nc.tensor.matmul(
                                sc_ps[:, t_sl], uh[:, et, s_sl],
                                xh[:, et, t_sl],
                                start=(et == 0), stop=False)
                            nc.tensor.matmul(
                                sc_ps[:, t_sl], u8[:, et, :, s_sl],
                                x8[:, et, :, t_sl],
                                start=False, stop=(et == ED - 1),
                                perf_mode=DR)
                    negmax = small.tile([P, 1], f32, tag="negmax")
                    nc.vector.tensor_reduce(
                        negmax[:], sc_ps[:], axis=mybir.AxisListType.X,
                        op=mybir.AluOpType.max, negate=True)
                    bias8 = small.tile([P, 1], f32, tag="bias8")
                    nc.vector.tensor_scalar_mul(bias8[:], negmax[:], SCALE)
                    ptile = work.tile([P, S], f16, tag="p")
                    sumexp = small.tile([P, 1], f32, tag="sumexp")
                    nc.scalar.activation(
                        ptile[:], sc_ps[:], AF.Exp,
                        bias=bias8[:], scale=SCALE, accum_out=sumexp[:])
                    recip = small.tile([P, 1], f32, tag="recip")
                    nc.vector.reciprocal(recip[:], sumexp[:])
                    nc.vector.tensor_scalar_mul(ptile[:], ptile[:], recip[:])
                    pending.append((st, ptile))
                    if len(pending) == 2:
                        emit_transposes(*pending.pop(0))

                # ---- out[s,n] += sum_t P[s,t] w[t,n], accumulated over heads
                # The last s-tile's transposes are emitted after the first
                # out-group's tt=0..6 matmuls so the PE FIFO isn't blocked
                # behind softmax(st=7) latency (tt=7 is the only dependent).
                last_tp = pending.pop(0)
                for st in range(SD):
                    s_sl = slice(st * P, (st + 1) * P)
                    for (n0, n1) in ((0, 512), (512, 768)):
                        pr = mmps.tile([P, 512], f32, tag="mm512")
                        for tt in range(SD):
                            if last_tp is not None and tt == SD - 1:
                                emit_transposes(*last_tp)
                                last_tp = None
                            nc.tensor.matmul(
                                pr[:, :n1 - n0], pT[:, tt, s_sl],
                                wsb[:, tt, n0:n1],
                                start=(tt == 0), stop=(tt == SD - 1))
                        if h == 0:
                            nc.vector.tensor_copy(
                                acc[:, st, n0:n1], pr[:, :n1 - n0])
                        else:
                            nc.vector.tensor_add(
                                out=acc[:, st, n0:n1], in0=acc[:, st, n0:n1],
                                in1=pr[:, :n1 - n0])
                            if h == n_heads - 1 and n0 == 512:
                                # stream the finished s-tile out during the
                                # last head's remaining compute
                                nc.sync.dma_start(
                                    out_t[:, st, :], acc[:, st, :])

    nc.compile()
    return nc


def _get_nc():
    if "nc" not in _CACHE:
        _CACHE["nc"] = _build_nc()
    return _CACHE["nc"]


def _prepare(x, Wq, bq, Wk, bk, Wv, bv, Wp, bp):
    f16 = np.float16
    e5 = ml_dtypes.float8_e5m2
    x = np.asarray(x, dtype=np.float32)
    Wq = np.asarray(Wq, dtype=np.float32)
    Wk = np.asarray(Wk, dtype=np.float32)
    Wv = np.asarray(Wv, dtype=np.float32)
    Wp = np.asarray(Wp, dtype=np.float32)
    bq = np.asarray(bq, dtype=np.float32)
    bv = np.asarray(bv, dtype=np.float32)
    bp = np.asarray(bp, dtype=np.float32)

    # scores = x M x^T + ones (x r)^T with M = Wq Wk^T, r = Wk bq.
    # (x Wq bk^T and bq.bk shift rows uniformly and cancel in softmax.)
    M = np.matmul(Wq, Wk.transpose(0, 2, 1))          # [H, D, D]
    r = np.matmul(Wk, bq[:, :, None])[:, :, 0]        # [H, D]
    wp3 = Wp.reshape(H, D, D)
    W2 = np.matmul(Wv, wp3)                           # [H, D, D]

    # bv contributes sum_h bv_h @ Wp_h to every output row (softmax rows sum
    # to 1); fold it and bp into one host-side bias.  bk dropped entirely.
    bp_eff = (bp.astype(np.float64)
              + np.einsum('hd,hde->e', bv.astype(np.float64),
                          wp3.astype(np.float64))).astype(np.float32)

    M32 = MSCALE * M
    m_hi = M32.astype(f16)
    m8 = np.empty((H, D, 2, D), dtype=e5)
    m8[:, :, 0, :] = (M32 - m_hi.astype(np.float32)).astype(e5)  # lo (x full)
    m8[:, :, 1, :] = M32.astype(e5)                              # hi (x lo)

    shared = {
        "m_hi": m_hi, "m8": m8,
        "w2": W2.astype(f16),
        "r": (MSCALE * r).astype(np.float32),
    }
    in_maps = []
    for b in range(B):
        xT = np.ascontiguousarray(x[b].T)
        xt_hi = xT.astype(f16)
        x8 = np.empty((D, 2, S), dtype=e5)
        x8[:, 0, :] = xT.astype(e5)                              # full
        x8[:, 1, :] = (xT - xt_hi.astype(np.float32)).astype(e5)  # lo
        in_maps.append({"xT_hi": xt_hi, "x8": x8, **shared})
    return in_maps, bp_eff


def kernel(x, Wq, bq, Wk, bk, Wv, bv, Wp, bp):
    from concourse.bass_utils import run_bass_kernel_spmd

    in_maps, bp_eff = _prepare(x, Wq, bq, Wk, bk, Wv, bv, Wp, bp)
    nc = _get_nc()
    res = run_bass_kernel_spmd(nc, in_maps, list(range(B)))
    out = np.stack([res.results[b]["out"] for b in range(B)], axis=0)
    out = out + bp_eff[None, None, :]
    return out.astype(np.float32)


# revision 26
# speedup vs baseline: 2.3158x; 1.0088x over previous
"""Trainium2 Bass kernel for nn_MultiHeadAttention_65352222376626.

Reference computation (B=8, S=1024, D=768, H=12):
    q = einsum('bsd,hde->bhse', x, Wq) + bq      # per-head full-width projections
    k, v likewise
    scores = einsum('bhse,bhte->bhst', q, k) * sqrt(64)
    attn = softmax(scores, -1)
    o = einsum('bhst,bhte->bhse', attn, v)
    out = concat_heads(o) @ Wp + bp

Sharding: pure batch-parallel - B == n_cores == 8, one batch element per
NeuronCore, full weights replicated per core.  No collectives needed.

Algebraic restructure: since softmax is row-shift invariant,
    scores = x (Wq Wk^T) x^T + ones_s (x Wk bq)^T   [+ row-const terms dropped]
so we precompute M_h = Wq_h @ Wk_h^T and r_h = Wk_h @ bq_h on the host and
replace the q-proj + k-proj + scores pipeline (3 big matmuls) with
u = x@M + r followed by scores = u @ x^T (2 big matmuls).  Likewise by
associativity (P@v)@Wp = P@(x@(Wv@Wp)), so W2_h = Wv_h @ Wp_h is
precomputed and the v-proj + attn@v + out-proj trio becomes
w = x@W2 then out += P@w (2 big matmuls).

Numerics: the softmax is near-argmax (score std ~222, top-2 gaps ~60), so
logit errors flip argmaxes and blow up the absmax metric; the u-proj and
score matmul operands need ~16+ mantissa bits.  Each runs as one fp16
hi*hi pass plus ONE fp8-e5m2 DoubleRow matmul that computes both
correction terms (lo*full + hi*lo) as a K-interleaved pair at 2x rate -
1.5 effective passes instead of 3.  The correction terms are ~2^-11 of
the main term, so 3-bit e5m2 mantissas suffice (validated: absmax rel
err 2.3e-3 vs fp32 reference).  M is pre-scaled by 32 on the host so its
fp8 splits stay in e5m2's normal range; the 1/32 is folded into the
PSUM->SBUF epilogue.  The w/out path is tolerance-insensitive and runs
single-pass fp16.  bk shifts score rows by a constant and cancels in
softmax; bv's contribution is sum_h bv_h @ Wp_h (softmax rows sum to 1),
folded with bp into one host-side bias add.
"""

import numpy as np
import ml_dtypes

B, S, D, H = 8, 1024, 768, 12
P = 128
SD = S // P   # 8 tiles along the sequence axis
ED = D // P   # 6 tiles along the feature axis
SCALE = 8.0   # sqrt(head_dim=64); reference multiplies scores by this
MSCALE = 32.0  # host pre-scale of M so fp8 splits stay in e5m2 range

_CACHE = {}


def _build_nc(n_heads=H):
    import concourse.tile as tile
    from concourse import bacc, mybir
    from concourse.masks import make_identity

    f32 = mybir.dt.float32
    f16 = mybir.dt.float16
    f8 = mybir.dt.float8e5
    AF = mybir.ActivationFunctionType
    ALU = mybir.AluOpType
    DR = mybir.MatmulPerfMode.DoubleRow

    nc = bacc.Bacc()

    # ---- DRAM I/O (per core) ----
    xh_d = nc.dram_tensor("xT_hi", [D, S], f16, kind="ExternalInput")
    x8_d = nc.dram_tensor("x8", [D, 2, S], f8, kind="ExternalInput")
    mh_d = nc.dram_tensor("m_hi", [H, D, D], f16, kind="ExternalInput")
    m8_d = nc.dram_tensor("m8", [H, D, 2, D], f8, kind="ExternalInput")
    w2_d = nc.dram_tensor("w2", [H, D, D], f16, kind="ExternalInput")
    r_d = nc.dram_tensor("r", [H, D], f32, kind="ExternalInput")
    out_d = nc.dram_tensor("out", [S, D], f32, kind="ExternalOutput")

    # partition-tiled DRAM views
    xh_t = xh_d.rearrange("(o p) s -> p o s", p=P)          # [128, ED, S]
    x8_t = x8_d.rearrange("(o p) t s -> p o t s", p=P)      # [128, ED, 2, S]
    mh_t = mh_d.rearrange("h (o p) e -> h p o e", p=P)      # [H, 128, ED, D]
    m8_t = m8_d.rearrange("h (o p) t e -> h p o t e", p=P)  # [H, 128, ED, 2, D]
    w2_t = w2_d.rearrange("h (o p) e -> h p o e", p=P)
    r_t = r_d.rearrange("h (o p) -> p h o", p=P)            # [128, H, ED]
    out_t = out_d.rearrange("(o p) d -> p o d", p=P)        # [128, SD, D]

    with tile.TileContext(nc) as tc:
        with (
            tc.tile_pool(name="persist", bufs=1) as persist,
            tc.tile_pool(name="whead", bufs=2) as whead,
            tc.tile_pool(name="work", bufs=2) as work,
            tc.tile_pool(name="small", bufs=4) as small,
            tc.tile_pool(name="mmps", bufs=2, space="PSUM") as mmps,
            tc.tile_pool(name="tpps", bufs=2, space="PSUM") as tpps,
            tc.tile_pool(name="scps", bufs=2, space="PSUM") as scps,
        ):
            # ---- persistent tiles ----
            # startup critical path: head 0 runs w-proj first (needs only
            # xh + w2), so order the DMA queue xh, w2, then the u-proj
            # inputs, which stream in while the w-projection computes
            # chunked so head 0's first w-proj group starts after ~0.7MB of
            # DMA instead of the full 2.7MB (xh then w2 serially)
            xh = persist.tile([P, ED, S], f16)
            w2first = whead.tile([P, ED, D], f16, tag="w2", name="w2_0")
            nc.sync.dma_start(xh[:, :, 0:P], xh_t[:, :, 0:P])
            nc.sync.dma_start(w2first[:, :, 0:512], w2_t[0][:, :, 0:512])
            nc.sync.dma_start(w2first[:, :, 512:D], w2_t[0][:, :, 512:D])
            nc.sync.dma_start(xh[:, :, P:512], xh_t[:, :, P:512])
            nc.sync.dma_start(xh[:, :, 512:S], xh_t[:, :, 512:S])
            mnext = [whead.tile([P, ED, D], f16, tag="mh", name="mhi0"),
                     whead.tile([P, ED, 2, D], f8, tag="m8", name="m8_0")]
            nc.sync.dma_start(mnext[0][:], mh_t[0])
            nc.sync.dma_start(mnext[1][:], m8_t[0])
            x8 = persist.tile([P, ED, 2, S], f8)
            nc.sync.dma_start(x8[:], x8_t)
            rsb = persist.tile([P, H, ED], f32)
            nc.sync.dma_start(rsb[:], r_t)
            ident = persist.tile([P, P], f16)
            make_identity(nc, ident)
            acc = persist.tile([P, SD, D], f32)     # final accumulator

            for h in range(n_heads):
                mhi, m8h = mnext
                if h == 0:
                    w2h = w2first
                else:
                    w2h = whead.tile([P, ED, D], f16, tag="w2")
                    nc.sync.dma_start(w2h[:], w2_t[h])

                def emit_uproj(h, mhi, m8h, uh, u8):
                    # u[e,s] = (sum_d 32M[d,e] xT[d,s] + 32r[e])/32
                    # fp16 hi*hi pass + one DoubleRow fp8 pass for (lo*full
                    # + hi*lo); epilogue writes fp16 hi + fp8 (lo, hi)
                    # operand set for the score matmul.
                    for et in range(ED):
                        e_sl = slice(et * P, (et + 1) * P)
                        for sc_ in range(2):
                            s_sl = slice(sc_ * 512, (sc_ + 1) * 512)
                            if et == 0:
                                # first two groups borrow score-pool banks
                                # (idle at head start) so they needn't wait
                                # for the previous head's out-accum epilogue
                                ps = scps.tile(
                                    [P, S], f32, tag="sc",
                                    name=f"ups{sc_}")[:, :512]
                            else:
                                ps = mmps.tile([P, 512], f32, tag="mm512")
                            for dt_ in range(ED):
                                nc.tensor.matmul(
                                    ps[:], mhi[:, dt_, e_sl],
                                    xh[:, dt_, s_sl],
                                    start=(dt_ == 0), stop=False)
                                nc.tensor.matmul(
                                    ps[:], m8h[:, dt_, :, e_sl],
                                    x8[:, dt_, :, s_sl],
                                    start=False, stop=(dt_ == ED - 1),
                                    perf_mode=DR)
                            # ps = (ps + 32r)/32, then fp16 hi + fp8 pair
                            nc.vector.tensor_scalar(
                                ps[:], ps[:], rsb[:, h, et:et + 1],
                                1.0 / MSCALE, op0=ALU.add, op1=ALU.mult)
                            nc.scalar.activation(
                                uh[:, et, s_sl], ps[:], AF.Copy)
                            nc.scalar.activation(
                                u8[:, et, 1, s_sl], ps[:], AF.Copy)
                            nc.vector.tensor_sub(
                                u8[:, et, 0, s_sl], ps[:], uh[:, et, s_sl])

                def emit_wproj(w2h, wsb):
                    # w[t,n] = sum_d xT[d,t] W2[d,n], single-pass fp16
                    for tt in range(SD):
                        t_sl = slice(tt * P, (tt + 1) * P)
                        for (n0, n1) in ((0, 512), (512, 768)):
                            ps = mmps.tile([P, 512], f32, tag="mm512")
                            for dt_ in range(ED):
                                nc.tensor.matmul(
                                    ps[:, :n1 - n0], xh[:, dt_, t_sl],
                                    w2h[:, dt_, n0:n1],
                                    start=(dt_ == 0), stop=(dt_ == ED - 1))
                            nc.scalar.activation(
                                wsb[:, tt, n0:n1], ps[:, :n1 - n0], AF.Copy)

                uh = work.tile([P, ED, S], f16, tag="uh", bufs=1)
                u8 = work.tile([P, ED, 2, S], f8, tag="u8", bufs=1)
                wsb = work.tile([P, SD, D], f16, tag="w", bufs=1)
                if h == 0:
                    # w-proj first: it only needs xh + w2 (2.7MB of DMA) so
                    # the PE starts while x8/m_hi/m8 are still streaming in
                    emit_wproj(w2h, wsb)
                    emit_uproj(h, mhi, m8h, uh, u8)
                else:
                    emit_uproj(h, mhi, m8h, uh, u8)

                # prefetch next head's M during this head's compute
                if h + 1 < n_heads:
                    mnext = [
                        whead.tile([P, ED, D], f16, tag="mh",
                                   name=f"mhi{h + 1}"),
                        whead.tile([P, ED, 2, D], f8, tag="m8",
                                   name=f"m8_{h + 1}")]
                    nc.sync.dma_start(mnext[0][:], mh_t[h + 1])
                    nc.sync.dma_start(mnext[1][:], m8_t[h + 1])

                if h > 0:
                    emit_wproj(w2h, wsb)

                # ---- scores + softmax; transposes one s-tile behind ----
                pT = work.tile([P, SD, S], f16, tag="pT", bufs=1)

                def emit_transposes(st, ptile):
                    s_sl = slice(st * P, (st + 1) * P)
                    for g in range(2):
                        tp_ps = tpps.tile([P, 4, P], f16, tag="tp")
                        for k in range(4):
                            tt = g * 4 + k
                            nc.tensor.transpose(
                                tp_ps[:, k, :], ptile[:, tt * P:(tt + 1) * P],
                                ident[:])
                        nc.vector.tensor_copy(
                            pT[:, g * 4:(g + 1) * 4, s_sl], tp_ps[:])

                pending = []
                for st in range(SD):
                    s_sl = slice(st * P, (st + 1) * P)
                    sc_ps = scps.tile([P, S], f32, tag="sc")
                    for tch in range(2):
                        t_sl = slice(tch * 512, (tch + 1) * 512)
                        for et in range(ED):
                            nc.tensor.matmul(
                                sc_ps[:, t_sl], uh[:, et, s_sl],
                                xh[:, et, t_sl],
                                start=(et == 0), stop=False)
                            nc.tensor.matmul(
                                sc_ps[:, t_sl], u8[:, et, :, s_sl],
                                x8[:, et, :, t_sl],
                                start=False, stop=(et == ED - 1),
                                perf_mode=DR)
                    negmax = small.tile([P, 1], f32, tag="negmax")
                    nc.vector.tensor_reduce(
                        negmax[:], sc_ps[:], axis=mybir.AxisListType.X,
                        op=mybir.AluOpType.max, negate=True)
                    bias8 = small.tile([P, 1], f32, tag="bias8")
                    nc.vector.tensor_scalar_mul(bias8[:], negmax[:], SCALE)
                    ptile = work.tile([P, S], f16, tag="p")
                    sumexp = small.tile([P, 1], f32, tag="sumexp")
                    nc.scalar.activation(
                        ptile[:], sc_ps[:], AF.Exp,
                        bias=bias8[:], scale=SCALE, accum_out=sumexp[:])
                    recip = small.tile([P, 1], f32, tag="recip")
                    nc.vector.reciprocal(recip[:], sumexp[:])
                    nc.vector.tensor_scalar_mul(ptile[:], ptile[:], recip[:])
                    pending.append((st, ptile))
                    if len(pending) == 2:
                        emit_transposes(*pending.pop(0))

                # ---- out[s,n] += sum_t P[s,t] w[t,n], accumulated over heads
                # The last s-tile's transposes are emitted after the first
                # out-group's tt=0..6 matmuls so the PE FIFO isn't blocked
                # behind softmax(st=7) latency (tt=7 is the only dependent).
                last_tp = pending.pop(0)
                for st in range(SD):
                    s_sl = slice(st * P, (st + 1) * P)
                    for (n0, n1) in ((0, 512), (512, 768)):
                        pr = mmps.tile([P, 512], f32, tag="mm512")
                        for tt in range(SD):
                            if last_tp is not None and tt == SD - 1:
                                emit_transposes(*last_tp)
                                last_tp = None
                            nc.tensor.matmul(
                                pr[:, :n1 - n0], pT[:, tt, s_sl],
                                wsb[:, tt, n0:n1],
                                start=(tt == 0), stop=(tt == SD - 1))
                        if h == 0:
                            nc.vector.tensor_copy(
                                acc[:, st, n0:n1], pr[:, :n1 - n0])
                        else:
                            nc.vector.tensor_add(
                                out=acc[:, st, n0:n1], in0=acc[:, st, n0:n1],
                                in1=pr[:, :n1 - n0])
                            if h == n_heads - 1 and n0 == 512:
                                # stream the finished s-tile out during the
                                # last head's remaining compute
                                nc.sync.dma_start(
                                    out_t[:, st, :], acc[:, st, :])

    nc.compile()
    return nc


def _get_nc():
    if "nc" not in _CACHE:
        _CACHE["nc"] = _build_nc()
    return _CACHE["nc"]


def _prepare(x, Wq, bq, Wk, bk, Wv, bv, Wp, bp):
    f16 = np.float16
    e5 = ml_dtypes.float8_e5m2
    x = np.asarray(x, dtype=np.float32)
    Wq = np.asarray(Wq, dtype=np.float32)
    Wk = np.asarray(Wk, dtype=np.float32)
    Wv = np.asarray(Wv, dtype=np.float32)
    Wp = np.asarray(Wp, dtype=np.float32)
    bq = np.asarray(bq, dtype=np.float32)
    bv = np.asarray(bv, dtype=np.float32)
    bp = np.asarray(bp, dtype=np.float32)

    # scores = x M x^T + ones (x r)^T with M = Wq Wk^T, r = Wk bq.
    # (x Wq bk^T and bq.bk shift rows uniformly and cancel in softmax.)
    M = np.matmul(Wq, Wk.transpose(0, 2, 1))          # [H, D, D]
    r = np.matmul(Wk, bq[:, :, None])[:, :, 0]        # [H, D]
    wp3 = Wp.reshape(H, D, D)
    W2 = np.matmul(Wv, wp3)                           # [H, D, D]

    # bv contributes sum_h bv_h @ Wp_h to every output row (softmax rows sum
    # to 1); fold it and bp into one host-side bias.  bk dropped entirely.
    bp_eff = (bp.astype(np.float64)
              + np.einsum('hd,hde->e', bv.astype(np.float64),
                          wp3.astype(np.float64))).astype(np.float32)

    M32 = MSCALE * M
    m_hi = M32.astype(f16)
    m8 = np.empty((H, D, 2, D), dtype=e5)
    m8[:, :, 0, :] = (M32 - m_hi.astype(np.float32)).astype(e5)  # lo (x full)
    m8[:, :, 1, :] = M32.astype(e5)                              # hi (x lo)

    shared = {
        "m_hi": m_hi, "m8": m8,
        "w2": W2.astype(f16),
        "r": (MSCALE * r).astype(np.float32),
    }
    in_maps = []
    for b in range(B):
        xT = np.ascontiguousarray(x[b].T)
        xt_hi = xT.astype(f16)
        x8 = np.empty((D, 2, S), dtype=e5)
        x8[:, 0, :] = xT.astype(e5)                              # full
        x8[:, 1, :] = (xT - xt_hi.astype(np.float32)).astype(e5)  # lo
        in_maps.append({"xT_hi": xt_hi, "x8": x8, **shared})
    return in_maps, bp_eff


def kernel(x, Wq, bq, Wk, bk, Wv, bv, Wp, bp):
    from concourse.bass_utils import run_bass_kernel_spmd

    in_maps, bp_eff = _prepare(x, Wq, bq, Wk, bk, Wv, bv, Wp, bp)
    nc = _get_nc()
    res = run_bass_kernel_spmd(nc, in_maps, list(range(B)))
    out = np.stack([res.results[b]["out"] for b in range(B)], axis=0)
    out = out + bp_eff[None, None, :]
    return out.astype(np.float32)
